# revision 1
# baseline (speedup 1.0000x reference)
"""nn_BoundaryGuidedDSTLayer Trainium2 Bass kernel (8-core SPMD, no collectives).

Sharding: core c = (b = c//2, half = c%2). Each core computes the conv
pre-mix + LN1 + K/V over the full T of its batch (needed for full
attention), and Q / attention / out-proj / MLP / DSA only for its local
1024-column half. All activations live transposed [C, T] so chained
matmuls contract over the partition dim in fp32r at full PE rate.
"""
import sys, os

for _p in ("/opt/trn_rl_repo",):
    if os.path.isdir(_p) and _p not in sys.path:
        sys.path.append(_p)

import numpy as np
import concourse.bass as bass
import concourse.mybir as mybir
import concourse.tile as tile
from concourse.bacc import Bacc
from concourse.bass_utils import run_bass_kernel_spmd

dt = mybir.dt
F32, F32R, U32 = dt.float32, dt.float32r, dt.uint32
AF = mybir.ActivationFunctionType
OP = mybir.AluOpType

P = 128
B, T, C, H = 4, 2048, 512, 8
HD = C // H          # 64
FF = 4 * C           # 2048
TL = T // 2          # 1024 local columns per core
CK = C // P          # 4
FFK = FF // P        # 16
NCH = T // 512       # 4 chunks over full T
NL = TL // 512       # 2 chunks over local T
TK = T // P          # 16 key tiles

_CACHED = None


def _build():
    nc = Bacc("TRN2", target_bir_lowering=False, debug=False, num_devices=8)

    # ---- DRAM I/O ----
    d_xT = nc.dram_tensor("xT", [C, T + 2], F32, kind="ExternalInput")
    d_xd = nc.dram_tensor("xd", [C, TL + 2], F32, kind="ExternalInput")
    d_A = nc.dram_tensor("Arow", [1, T], F32, kind="ExternalInput")
    d_qA = nc.dram_tensor("qArow", [1, TL], F32, kind="ExternalInput")
    d_mask = nc.dram_tensor("maskbc", [P, TL + 2], F32, kind="ExternalInput")
    d_qoff = nc.dram_tensor("qoff", [1, 1], U32, kind="ExternalInput")
    d_convw = nc.dram_tensor("convw", [3, C, C], F32, kind="ExternalInput")
    d_convb = nc.dram_tensor("convb", [C], F32, kind="ExternalInput")
    d_wqkv = nc.dram_tensor("wqkv", [C, 3 * C], F32, kind="ExternalInput")
    d_bqkv = nc.dram_tensor("bqkv", [3 * C], F32, kind="ExternalInput")
    d_bvbc = nc.dram_tensor("bvbc", [P, C], F32, kind="ExternalInput")
    d_wo = nc.dram_tensor("wo", [HD, H, C], F32, kind="ExternalInput")
    d_ob = nc.dram_tensor("ob", [C], F32, kind="ExternalInput")
    d_w1 = nc.dram_tensor("w1", [C, FF], F32, kind="ExternalInput")
    d_b1 = nc.dram_tensor("b1", [FF], F32, kind="ExternalInput")
    d_w2 = nc.dram_tensor("w2", [FF, C], F32, kind="ExternalInput")
    d_bfin = nc.dram_tensor("bfin", [C], F32, kind="ExternalInput")
    d_pw = nc.dram_tensor("pw", [C, C], F32, kind="ExternalInput")
    d_dsag = nc.dram_tensor("dsag", [C], F32, kind="ExternalInput")
    d_dsab = nc.dram_tensor("dsab", [C], F32, kind="ExternalInput")
    d_dw3 = nc.dram_tensor("dw3", [C, 3], F32, kind="ExternalInput")
    d_dsadb = nc.dram_tensor("dsadb", [C], F32, kind="ExternalInput")
    d_ones = nc.dram_tensor("cones", [P, P], F32, kind="ExternalInput")
    d_invC = nc.dram_tensor("cinvC", [P, 1], F32, kind="ExternalInput")
    d_eps = nc.dram_tensor("ceps", [1, 1], F32, kind="ExternalInput")
    d_out = nc.dram_tensor("outT", [C, TL], F32, kind="ExternalOutput")

    eng = nc.vector  # DVE for elementwise

    with tile.TileContext(nc) as tc, nc.allow_low_precision(
            reason="float32r tiles are bit-identical to float32"):
        # ---------- persistent small pools ----------
        consts = tc.alloc_tile_pool(name="consts", bufs=1, side="left")
        ones_r = consts.tile([P, P], F32R, tag="ones")
        nc.sync.dma_start(out=ones_r, in_=d_ones[:, :].bitcast(F32R))
        invC_r = consts.tile([P, 1], F32R, tag="invC")
        nc.sync.dma_start(out=invC_r, in_=d_invC[:, :].bitcast(F32R))
        convb_s = consts.tile([P, CK], F32, tag="convb")
        nc.sync.dma_start(out=convb_s, in_=d_convb.rearrange("(m p) -> p m", p=P))
        bqkv_s = consts.tile([P, 12], F32, tag="bqkv")
        nc.sync.dma_start(out=bqkv_s, in_=d_bqkv.rearrange("(m p) -> p m", p=P))
        ob_s = consts.tile([P, CK], F32, tag="ob")
        nc.sync.dma_start(out=ob_s, in_=d_ob.rearrange("(m p) -> p m", p=P))
        b1_s = consts.tile([P, FFK], F32, tag="b1")
        nc.sync.dma_start(out=b1_s, in_=d_b1.rearrange("(m p) -> p m", p=P))
        bfin_s = consts.tile([P, CK], F32, tag="bfin")
        nc.sync.dma_start(out=bfin_s, in_=d_bfin.rearrange("(m p) -> p m", p=P))
        dsag_s = consts.tile([P, CK], F32, tag="dsag")
        nc.sync.dma_start(out=dsag_s, in_=d_dsag.rearrange("(m p) -> p m", p=P))
        dsab_s = consts.tile([P, CK], F32, tag="dsab")
        nc.sync.dma_start(out=dsab_s, in_=d_dsab.rearrange("(m p) -> p m", p=P))
        dw3_s = consts.tile([P, CK, 3], F32, tag="dw3")
        nc.sync.dma_start(out=dw3_s, in_=d_dw3.rearrange("(m p) d -> p m d", p=P))
        dsadb_s = consts.tile([P, CK], F32, tag="dsadb")
        nc.sync.dma_start(out=dsadb_s, in_=d_dsadb.rearrange("(m p) -> p m", p=P))
        bvbc_s = consts.tile([P, C], F32, tag="bvbc")
        nc.sync.dma_start(out=bvbc_s, in_=d_bvbc[:, :])
        eps_s = consts.tile([1, 1], F32, tag="eps")
        nc.sync.dma_start(out=eps_s, in_=d_eps[:, :])
        qoff_s = consts.tile([1, 1], U32, tag="qoff")
        nc.sync.dma_start(out=qoff_s, in_=d_qoff[:, :])
        regs = nc.alloc_registers("qoffr")
        nc.regs_load(regs, qoff_s[0:1, 0:1])
        j0 = nc.snap(regs, donate=True, min_val=0, max_val=TL)

        # ---------- persistent activation state ----------
        sq_pool = tc.alloc_tile_pool(name="sq", bufs=2, side="left")
        vec_pool = tc.alloc_tile_pool(name="vec", bufs=1, side="left")
        tmp_pool = tc.alloc_tile_pool(name="tmp", bufs=2, side="left")
        hat_pool = tc.alloc_tile_pool(name="hatp", bufs=1, side="left")
        hat = hat_pool.tile([P, CK, T], F32R, tag="hat")

        # =================== Phase A1: conv + LN1 -> hat ===================
        a1 = tc.alloc_tile_pool(name="a1", bufs=1, side="left")
        convw_s = a1.tile([P, 3, CK, C], F32R, tag="convw")
        nc.sync.dma_start(
            out=convw_s,
            in_=d_convw.rearrange("d (k p) o -> p d k o", p=P).bitcast(F32R),
        )
        xch_pool = tc.alloc_tile_pool(name="xch", bufs=3, side="left")
        ftc_pool = tc.alloc_tile_pool(name="ftc", bufs=2, side="left")
        psA = tc.alloc_tile_pool(name="psA", bufs=2, space="PSUM")
        psS = tc.alloc_tile_pool(name="psS", bufs=1, space="PSUM")
        psB = tc.alloc_tile_pool(name="psB", bufs=1, space="PSUM")

        def ln_stats_and_scale(psS, psB, src_tiles, n_cols, sq_tag):
            """src_tiles: list of CK [P, n_cols] f32r APs (one per kc).
            Returns psum tile [P, 2*n_cols]: [:, :n] = r_bc, [:, n:] = m*r_bc."""
            ps_mean = psS.tile([1, 512], F32, tag="mean")
            for kc in range(CK):
                nc.tensor.matmul(ps_mean[0:1, :n_cols], invC_r[:, :], src_tiles[kc],
                                 start=(kc == 0), stop=(kc == CK - 1))
            ps_ex2 = psS.tile([1, 512], F32, tag="ex2")
            for kc in range(CK):
                sq_t = sq_pool.tile([P, 512], F32R, tag="sq")
                nc.scalar.activation(out=sq_t[:, :n_cols], in_=src_tiles[kc].bitcast(F32),
                                     func=AF.Square)
                nc.tensor.matmul(ps_ex2[0:1, :n_cols], invC_r[:, :], sq_t[:, :n_cols],
                                 start=(kc == 0), stop=(kc == CK - 1))
            m_s = vec_pool.tile([1, 512], F32R, tag="m")
            eng.tensor_copy(out=m_s[:, :n_cols], in_=ps_mean[0:1, :n_cols])
            var_s = vec_pool.tile([1, 512], F32, tag="var")
            eng.tensor_tensor(out=var_s[:, :n_cols], in0=m_s[:, :n_cols].bitcast(F32),
                              in1=m_s[:, :n_cols].bitcast(F32), op=OP.mult)
            eng.tensor_tensor(out=var_s[:, :n_cols], in0=ps_ex2[0:1, :n_cols],
                              in1=var_s[:, :n_cols], op=OP.subtract)
            std_s = vec_pool.tile([1, 512], F32, tag="std")
            nc.scalar.activation(out=std_s[:, :n_cols], in_=var_s[:, :n_cols],
                                 func=AF.Sqrt, bias=eps_s[0:1, 0:1])
            r_s = vec_pool.tile([1, 512], F32R, tag="r")
            eng.reciprocal(out=r_s[:, :n_cols], in_=std_s[:, :n_cols])
            mr_s = vec_pool.tile([1, 512], F32R, tag="mr")
            eng.tensor_tensor(out=mr_s[:, :n_cols], in0=m_s[:, :n_cols].bitcast(F32),
                              in1=r_s[:, :n_cols].bitcast(F32), op=OP.mult)
            ps_bc = psB.tile([P, 1024], F32, tag="lnbc")
            nc.tensor.matmul(ps_bc[:, 0:n_cols], ones_r[0:1, :], r_s[:, :n_cols],
                             start=True, stop=True)
            nc.tensor.matmul(ps_bc[:, 512:512 + n_cols], ones_r[0:1, :], mr_s[:, :n_cols],
                             start=True, stop=True)
            return ps_bc

        for n in range(NCH):
            c0 = 512 * n
            x_ch = xch_pool.tile([P, CK, 514], F32R, tag="xch")
            nc.sync.dma_start(
                out=x_ch,
                in_=d_xT[:, c0:c0 + 514].rearrange("(k p) t -> p k t", p=P).bitcast(F32R),
            )
            ftc_t = []
            for mo in range(CK):
                ps_c = psA.tile([P, 512], F32, tag="mm")
                first = True
                for dtap in range(3):
                    for kc in range(CK):
                        nc.tensor.matmul(
                            ps_c,
                            convw_s[:, dtap, kc, mo * P:(mo + 1) * P],
                            x_ch[:, kc, dtap:dtap + 512],
                            start=first, stop=(dtap == 2 and kc == CK - 1),
                        )
                        first = False
                f_t = ftc_pool.tile([P, 512], F32R, tag=f"ftc{mo}")
                nc.scalar.activation(out=f_t, in_=ps_c, func=AF.Gelu,
                                     bias=convb_s[:, mo:mo + 1])
                eng.tensor_tensor(out=f_t, in0=f_t.bitcast(F32),
                                  in1=x_ch[:, mo, 1:513].bitcast(F32), op=OP.add)
                ftc_t.append(f_t)
            ps_bc = ln_stats_and_scale(psS, psB, ftc_t, 512, "sq")
            for kc in range(CK):
                t_s = tmp_pool.tile([P, 512], F32, tag="t")
                eng.tensor_tensor(out=t_s, in0=ftc_t[kc].bitcast(F32),
                                  in1=ps_bc[:, 0:512], op=OP.mult)
                eng.tensor_tensor(out=hat[:, kc, c0:c0 + 512], in0=t_s,
                                  in1=ps_bc[:, 512:1024], op=OP.subtract)
        for pool in (ftc_pool, xch_pool, a1):
            pool.release()

        # =================== Phase A2: K, V, Q ===================
        kv_state = tc.alloc_tile_pool(name="kvst", bufs=1, side="right")
        st_pool = tc.alloc_tile_pool(name="stage", bufs=2, side="right")
        a2 = tc.alloc_tile_pool(name="a2", bufs=1, side="right")
        wkv_s = a2.tile([P, CK, 2 * C], F32R, tag="wkv")
        nc.sync.dma_start(
            out=wkv_s,
            in_=d_wqkv.rearrange("(k p) o -> p k o", p=P)[:, :, C:3 * C].bitcast(F32R),
        )
        kaug = kv_state.tile([HD + 1, H, T], F32R, tag="kaug")
        qaug = kv_state.tile([HD + 1, H, TL], F32R, tag="qaug")
        vsb = kv_state.tile([P, TK, H, HD + 1], F32R, tag="v")

        # v ones column
        eng.tensor_copy(out=vsb[:, :, :, HD], in_=ones_r.rearrange("p (g h) -> p g h", h=H)[:, 0:TK, :].bitcast(F32))
        # aug rows
        for h in range(H):
            nc.sync.dma_start(out=kaug[HD:HD + 1, h, :], in_=d_A[:, :].bitcast(F32R))
            nc.sync.dma_start(out=qaug[HD:HD + 1, h, :], in_=d_qA[:, :].bitcast(F32R))

        for n in range(NCH):
            c0 = 512 * n
            # K tiles
            for mo in range(CK):
                ps_k = psA.tile([P, 512], F32, tag="mm")
                for kc in range(CK):
                    nc.tensor.matmul(ps_k, wkv_s[:, kc, mo * P:(mo + 1) * P],
                                     hat[:, kc, c0:c0 + 512],
                                     start=(kc == 0), stop=(kc == CK - 1))
                st = st_pool.tile([P, 512], F32R, tag="kst")
                eng.tensor_scalar(out=st, in0=ps_k, scalar1=bqkv_s[:, 4 + mo:5 + mo],
                                  scalar2=None, op0=OP.add)
                nc.sync.dma_start(out=kaug[0:HD, 2 * mo, c0:c0 + 512], in_=st[0:HD, :])
                nc.sync.dma_start(out=kaug[0:HD, 2 * mo + 1, c0:c0 + 512], in_=st[HD:P, :])
            # V tiles (natural layout)
            for tt in range(4):
                g = 4 * n + tt
                ps_v = psA.tile([P, 512], F32, tag="mm")
                for kc in range(CK):
                    nc.tensor.matmul(ps_v, hat[:, kc, c0 + tt * P:c0 + (tt + 1) * P],
                                     wkv_s[:, kc, C:2 * C],
                                     start=(kc == 0), stop=(kc == CK - 1))
                eng.tensor_tensor(out=vsb[:, g, :, 0:HD],
                                  in0=ps_v.rearrange("p (h d) -> p h d", d=HD),
                                  in1=bvbc_s.rearrange("p (h d) -> p h d", d=HD),
                                  op=OP.add)
        # Q tiles (local half via dynamic offset)
        a2.release()
        a2q = tc.alloc_tile_pool(name="a2q", bufs=1, side="right")
        wq_s = a2q.tile([P, CK, C], F32R, tag="wq")
        nc.sync.dma_start(
            out=wq_s,
            in_=d_wqkv.rearrange("(k p) o -> p k o", p=P)[:, :, 0:C].bitcast(F32R),
        )
        for mo in range(CK):
            for n2 in range(NL):
                ps_q = psA.tile([P, 512], F32, tag="mm")
                for kc in range(CK):
                    nc.tensor.matmul(ps_q, wq_s[:, kc, mo * P:(mo + 1) * P],
                                     hat[:, kc, bass.ds(j0 + n2 * 512, 512)],
                                     start=(kc == 0), stop=(kc == CK - 1))
                st = st_pool.tile([P, 512], F32R, tag="kst")
                eng.tensor_scalar(out=st, in0=ps_q, scalar1=bqkv_s[:, mo:mo + 1],
                                  scalar2=None, op0=OP.add)
                nc.sync.dma_start(out=qaug[0:HD, 2 * mo, n2 * 512:(n2 + 1) * 512],
                                  in_=st[0:HD, :])
                nc.sync.dma_start(out=qaug[0:HD, 2 * mo + 1, n2 * 512:(n2 + 1) * 512],
                                  in_=st[HD:P, :])
        for pool in (a2q, st_pool, hat_pool, psB, psS, psA):
            pool.release()

        # =================== Attention ===================
        attn_state = tc.alloc_tile_pool(name="attnst", bufs=1, side="left")
        attnh = attn_state.tile([HD, H, TL], F32R, tag="attnh")
        p_pool = tc.alloc_tile_pool(name="pp", bufs=2, side="right")
        psS2 = tc.alloc_tile_pool(name="psS2", bufs=2, space="PSUM")
        psAV = tc.alloc_tile_pool(name="psAV", bufs=2, space="PSUM")

        for h in range(H):
            ps_av = psAV.tile([HD + 1, 1024], F32, tag="av")
            for tk in range(TK):
                ps_s = psS2.tile([P, 1024], F32, tag="score")
                for n2 in range(NL):
                    nc.tensor.matmul(ps_s[:, n2 * 512:(n2 + 1) * 512],
                                     kaug[:, h, tk * P:(tk + 1) * P],
                                     qaug[:, h, n2 * 512:(n2 + 1) * 512],
                                     start=True, stop=True)
                p_t = p_pool.tile([P, 1024], F32R, tag="p")
                nc.scalar.activation(out=p_t, in_=ps_s, func=AF.Exp)
                for n2 in range(NL):
                    nc.tensor.matmul(ps_av[:, n2 * 512:(n2 + 1) * 512],
                                     vsb[:, tk, h, :],
                                     p_t[:, n2 * 512:(n2 + 1) * 512],
                                     start=(tk == 0), stop=(tk == TK - 1))
            for n2 in range(NL):
                cc = slice(n2 * 512, (n2 + 1) * 512)
                d_s = vec_pool.tile([1, 512], F32R, tag="d")
                eng.reciprocal(out=d_s, in_=ps_av[HD:HD + 1, cc])
                ps_b = psS2.tile([P, 1024], F32, tag="score")
                nc.tensor.matmul(ps_b[0:HD, 0:512], ones_r[0:1, 0:HD], d_s,
                                 start=True, stop=True)
                db_s = tmp_pool.tile([HD, 512], F32, tag="dbs")
                eng.tensor_copy(out=db_s, in_=ps_b[0:HD, 0:512])
                eng.tensor_tensor(out=attnh[:, h, cc], in0=ps_av[0:HD, cc],
                                  in1=db_s, op=OP.mult)
        for pool in (p_pool, kv_state, psAV, psS2):
            pool.release()

        # =================== out-proj + residual + LN2 ===================
        late = tc.alloc_tile_pool(name="late", bufs=1, side="right")
        wo_s = late.tile([HD, H, C], F32R, tag="wo")
        nc.sync.dma_start(out=wo_s, in_=d_wo[:, :, :].bitcast(F32R))
        xd_s = late.tile([P, CK, TL + 2], F32R, tag="xd")
        nc.sync.dma_start(out=xd_s,
                          in_=d_xd.rearrange("(k p) t -> p k t", p=P).bitcast(F32R))
        ftc2 = late.tile([P, CK, TL], F32R, tag="ftc2")
        w1_s = late.tile([P, CK, FF], F32R, tag="w1")
        nc.sync.dma_start(out=w1_s,
                          in_=d_w1.rearrange("(k p) o -> p k o", p=P).bitcast(F32R))
        w2_s = late.tile([P, FFK, C], F32R, tag="w2")
        nc.sync.dma_start(out=w2_s,
                          in_=d_w2.rearrange("(k p) o -> p k o", p=P).bitcast(F32R))
        pw_s = late.tile([P, CK, C], F32R, tag="pw")
        dsa_out = late.tile([P, CK, TL], F32, tag="dsaout")
        nc.sync.dma_start(out=pw_s,
                          in_=d_pw.rearrange("(k p) o -> p k o", p=P).bitcast(F32R))

        psC = tc.alloc_tile_pool(name="psC", bufs=2, space="PSUM")
        psS_l = tc.alloc_tile_pool(name="psSl", bufs=1, space="PSUM")
        psB_l = tc.alloc_tile_pool(name="psBl", bufs=1, space="PSUM")
        for mo in range(CK):
            for n2 in range(NL):
                cc = slice(n2 * 512, (n2 + 1) * 512)
                ps_o = psC.tile([P, 512], F32, tag="mm")
                for h in range(H):
                    nc.tensor.matmul(ps_o, wo_s[:, h, mo * P:(mo + 1) * P],
                                     attnh[:, h, cc],
                                     start=(h == 0), stop=(h == H - 1))
                eng.scalar_tensor_tensor(
                    out=ftc2[:, mo, cc], in0=ps_o, scalar=ob_s[:, mo:mo + 1],
                    in1=xd_s[:, mo, 1 + n2 * 512:1 + (n2 + 1) * 512].bitcast(F32),
                    op0=OP.add, op1=OP.add)
        for n2 in range(NL):
            cc = slice(n2 * 512, (n2 + 1) * 512)
            src = [ftc2[:, kc, cc] for kc in range(CK)]
            ps_bc = ln_stats_and_scale(psS_l, psB_l, src, 512, "sq")
            for kc in range(CK):
                t_s = tmp_pool.tile([P, 512], F32, tag="t")
                eng.tensor_tensor(out=t_s, in0=ftc2[:, kc, cc].bitcast(F32),
                                  in1=ps_bc[:, 0:512], op=OP.mult)
                eng.tensor_tensor(out=ftc2[:, kc, cc], in0=t_s,
                                  in1=ps_bc[:, 512:1024], op=OP.subtract)
        attn_state.release()

        # =================== DSA branch ===================
        dsa_pool = tc.alloc_tile_pool(name="dsap", bufs=1, side="right")
        mask_s = dsa_pool.tile([P, TL + 2], F32, tag="mask")
        nc.sync.dma_start(out=mask_s, in_=d_mask[:, :])
        z_s = dsa_pool.tile([P, CK, TL + 2], F32, tag="z")
        z1_s = dsa_pool.tile([P, CK, TL], F32R, tag="z1")


        for (c0, w) in ((0, 512), (512, 512), (1024, 2)):
            src = [xd_s[:, kc, c0:c0 + w] for kc in range(CK)]
            ps_bc = ln_stats_and_scale(psS_l, psB_l, src, w, "sq")
            for kc in range(CK):
                t_s = tmp_pool.tile([P, 512], F32, tag="t")
                eng.tensor_tensor(out=t_s[:, :w], in0=xd_s[:, kc, c0:c0 + w].bitcast(F32),
                                  in1=ps_bc[:, 0:w], op=OP.mult)
                eng.tensor_tensor(out=t_s[:, :w], in0=t_s[:, :w],
                                  in1=ps_bc[:, 512:512 + w], op=OP.subtract)
                eng.tensor_scalar(out=t_s[:, :w], in0=t_s[:, :w],
                                  scalar1=dsag_s[:, kc:kc + 1], scalar2=dsab_s[:, kc:kc + 1],
                                  op0=OP.mult, op1=OP.add)
                eng.tensor_tensor(out=z_s[:, kc, c0:c0 + w], in0=t_s[:, :w],
                                  in1=mask_s[:, c0:c0 + w], op=OP.mult)
        for pool in (psB_l, psS_l):
            pool.release()
        for kc in range(CK):
            eng.tensor_scalar(out=z1_s[:, kc, :], in0=z_s[:, kc, 0:TL],
                              scalar1=dw3_s[:, kc, 0:1], scalar2=None, op0=OP.mult)
            eng.scalar_tensor_tensor(out=z1_s[:, kc, :], in0=z_s[:, kc, 1:1 + TL],
                                     scalar=dw3_s[:, kc, 1:2],
                                     in1=z1_s[:, kc, :].bitcast(F32),
                                     op0=OP.mult, op1=OP.add)
            eng.scalar_tensor_tensor(out=z1_s[:, kc, :], in0=z_s[:, kc, 2:2 + TL],
                                     scalar=dw3_s[:, kc, 2:3],
                                     in1=z1_s[:, kc, :].bitcast(F32),
                                     op0=OP.mult, op1=OP.add)
            nc.scalar.activation(out=z1_s[:, kc, :], in_=z1_s[:, kc, :].bitcast(F32),
                                 func=AF.Gelu, bias=dsadb_s[:, kc:kc + 1])
        for mo in range(CK):
            for n2 in range(NL):
                cc = slice(n2 * 512, (n2 + 1) * 512)
                ps_d = psC.tile([P, 512], F32, tag="mm")
                for kc in range(CK):
                    nc.tensor.matmul(ps_d, pw_s[:, kc, mo * P:(mo + 1) * P],
                                     z1_s[:, kc, cc],
                                     start=(kc == 0), stop=(kc == CK - 1))
                eng.tensor_copy(out=dsa_out[:, mo, cc], in_=ps_d)
        dsa_pool.release()

        # =================== MLP + final combine ===================
        hh_pool = tc.alloc_tile_pool(name="hh", bufs=3, side="left")
        fin_pool = tc.alloc_tile_pool(name="fin", bufs=3, side="left")
        psO = tc.alloc_tile_pool(name="psO", bufs=1, space="PSUM")
        for n2 in range(NL):
            cc = slice(n2 * 512, (n2 + 1) * 512)
            ps_out = [psO.tile([P, 512], F32, tag=f"out{mo}", name=f"psout{mo}") for mo in range(CK)]
            for ff in range(FFK):
                ps_h = psC.tile([P, 512], F32, tag="mm")
                for kc in range(CK):
                    nc.tensor.matmul(ps_h, w1_s[:, kc, ff * P:(ff + 1) * P],
                                     ftc2[:, kc, cc],
                                     start=(kc == 0), stop=(kc == CK - 1))
                hh_t = hh_pool.tile([P, 512], F32R, tag="hh")
                nc.scalar.activation(out=hh_t, in_=ps_h, func=AF.Gelu,
                                     bias=b1_s[:, ff:ff + 1])
                for mo in range(CK):
                    nc.tensor.matmul(ps_out[mo], w2_s[:, ff, mo * P:(mo + 1) * P],
                                     hh_t, start=(ff == 0), stop=(ff == FFK - 1))
            for mo in range(CK):
                fin_t = fin_pool.tile([P, 512], F32, tag="fin")
                eng.scalar_tensor_tensor(out=fin_t, in0=ps_out[mo],
                                         scalar=bfin_s[:, mo:mo + 1],
                                         in1=dsa_out[:, mo, cc],
                                         op0=OP.add, op1=OP.add)
                nc.sync.dma_start(out=d_out[mo * P:(mo + 1) * P, cc], in_=fin_t)

        for pool in (fin_pool, hh_pool, late, tmp_pool, vec_pool,
                     sq_pool, consts, psO, psC):
            pool.release()

    nc.compile()
    return nc


def _in_maps(inputs):
    f = lambda v: np.ascontiguousarray(np.asarray(v), dtype=np.float32)
    x = f(inputs["x"])            # [B, T, C]
    A = f(inputs["A"])            # [B, T]
    alpha = float(np.asarray(inputs["alpha_bias"]).reshape(-1)[0])
    dst_a = float(np.asarray(inputs["dst_alpha"]))
    dst_b = float(np.asarray(inputs["dst_beta"]))
    conv1_w, conv1_b = f(inputs["conv1_w"]), f(inputs["conv1_b"])
    ln1_g, ln1_b = f(inputs["ln1_g"]), f(inputs["ln1_b"])
    in_w, in_b = f(inputs["in_proj_w"]), f(inputs["in_proj_b"])
    out_w, out_b = f(inputs["out_w"]), f(inputs["out_b"])
    ln2_g, ln2_b = f(inputs["ln2_g"]), f(inputs["ln2_b"])
    w1, b1 = f(inputs["mlp_w1"]), f(inputs["mlp_b1"])
    w2, b2 = f(inputs["mlp_w2"]), f(inputs["mlp_b2"])
    dsa_g, dsa_b = f(inputs["dsa_ln_g"]), f(inputs["dsa_ln_b"])
    dsa_dw, dsa_db = f(inputs["dsa_dw"]), f(inputs["dsa_db"])
    dsa_pw, dsa_pb = f(inputs["dsa_pw"]), f(inputs["dsa_pb"])

    weff = in_w * ln1_g[None, :]
    beff = in_w @ ln1_b + in_b
    weff[:C] /= np.sqrt(HD).astype(np.float32)
    beff[:C] /= np.sqrt(HD).astype(np.float32)
    shared = {
        "convw": np.ascontiguousarray(np.transpose(conv1_w, (2, 1, 0))),
        "convb": conv1_b,
        "wqkv": np.ascontiguousarray(weff.T),
        "bqkv": beff,
        "bvbc": np.ascontiguousarray(np.broadcast_to(beff[2 * C:3 * C], (P, C))),
        "wo": np.ascontiguousarray(out_w.T.reshape(HD * H, C).reshape(H, HD, C).transpose(1, 0, 2)),
        "ob": out_b,
        "w1": np.ascontiguousarray((w1 * ln2_g[None, :]).T),
        "b1": w1 @ ln2_b + b1,
        "w2": np.ascontiguousarray((dst_a * w2).T),
        "bfin": dst_a * b2 + dst_b * dsa_pb,
        "pw": np.ascontiguousarray((dst_b * dsa_pw[:, :, 0]).T),
        "dsag": dsa_g, "dsab": dsa_b,
        "dw3": dsa_dw[:, 0, :], "dsadb": dsa_db,
        "cones": np.ones((P, P), np.float32),
        "cinvC": np.full((P, 1), 1.0 / C, np.float32),
        "ceps": np.full((1, 1), 1e-5, np.float32),
    }
    maps = []
    for core in range(8):
        b, half = core // 2, core % 2
        j0 = half * TL
        xT = np.zeros((C, T + 2), np.float32)
        xT[:, 1:T + 1] = x[b].T
        xd = np.zeros((C, TL + 2), np.float32)
        lo, hi = j0 - 1, j0 + TL + 1
        slo, shi = max(lo, 0), min(hi, T)
        xd[:, slo - lo:slo - lo + (shi - slo)] = x[b].T[:, slo:shi]
        mask = np.ones((1, TL + 2), np.float32)
        if lo < 0:
            mask[0, 0] = 0.0
        if hi > T:
            mask[0, TL + 1] = 0.0
        m = dict(shared)
        m["xT"] = xT
        m["xd"] = xd
        m["maskbc"] = np.ascontiguousarray(np.broadcast_to(mask, (P, TL + 2)))
        m["Arow"] = A[b:b + 1, :]
        m["qArow"] = alpha * A[b:b + 1, j0:j0 + TL]
        m["qoff"] = np.array([[j0]], np.uint32)
        maps.append(m)
    return maps


def _get_program():
    global _CACHED
    if _CACHED is None:
        _CACHED = _build()
    return _CACHED


def kernel(**inputs):
    nc = _get_program()
    maps = _in_maps(inputs)
    res = run_bass_kernel_spmd(nc, maps, list(range(8)))
    out = np.empty((B, T, C), np.float32)
    for core in range(8):
        b, half = core // 2, core % 2
        out[b, half * TL:(half + 1) * TL, :] = res.results[core]["outT"].T
    return out



# revision 31
# speedup vs baseline: 1.1465x; 1.1465x over previous
"""nn_BoundaryGuidedDSTLayer Trainium2 Bass kernel (8-core SPMD, no collectives).

Sharding: core c = (b = c//2, half = c%2). Each core computes the conv
pre-mix + LN1 + K/V over the full T of its batch (needed for full
attention), and Q / attention / out-proj / MLP / DSA only for its local
1024-column half. All activations live transposed [C, T].

v2: bf16 matmuls throughout (fp32 PSUM accumulate), fp32 kept for LN
stat rows / residual / final combine. ACT-table thrash minimized by
grouping sqrt sites; DVE reciprocal -> reciprocal_approx_fast.
"""
import sys, os

for _p in ("/opt/trn_rl_repo",):
    if os.path.isdir(_p) and _p not in sys.path:
        sys.path.append(_p)

import numpy as np
import ml_dtypes
import concourse.bass as bass
import concourse.mybir as mybir
import concourse.tile as tile
from concourse.bacc import Bacc
from concourse.bass_utils import run_bass_kernel_spmd

dt = mybir.dt
F32, F32R, U32 = dt.float32, dt.float32r, dt.uint32
BF, F16 = dt.bfloat16, dt.float16
AF = mybir.ActivationFunctionType
OP = mybir.AluOpType

P = 128
B, T, C, H = 4, 2048, 512, 8
HD = C // H          # 64
FF = 4 * C           # 2048
TL = T // 2          # 1024 local columns per core
CK = C // P          # 4
FFK = FF // P        # 16
NCH = T // 512       # 4 chunks over full T
NL = TL // 512       # 2 chunks over local T
TK = T // P          # 16 key tiles

_CACHED = None


def _build():
    nc = Bacc("TRN2", target_bir_lowering=False, debug=False, num_devices=8)

    # ---- DRAM I/O ----
    d_xT = nc.dram_tensor("xT", [C, T + 2], BF, kind="ExternalInput")
    d_xd = nc.dram_tensor("xd", [C, TL + 2], F32, kind="ExternalInput")
    d_xdb = nc.dram_tensor("xdb", [C, TL + 2], BF, kind="ExternalInput")
    d_A = nc.dram_tensor("Arow", [1, T], BF, kind="ExternalInput")
    d_qA = nc.dram_tensor("qArow", [1, TL], BF, kind="ExternalInput")
    d_qoff = nc.dram_tensor("qoff", [1, 1], U32, kind="ExternalInput")
    d_convw = nc.dram_tensor("convw", [3, C, C], BF, kind="ExternalInput")
    d_convb = nc.dram_tensor("convb", [C], F32, kind="ExternalInput")
    d_wqkv = nc.dram_tensor("wqkv", [C, 3 * C], BF, kind="ExternalInput")
    d_bqkv = nc.dram_tensor("bqkv", [3 * C], F32, kind="ExternalInput")
    d_bvbc = nc.dram_tensor("bvbc", [P, C], F32, kind="ExternalInput")
    d_wo = nc.dram_tensor("wo", [HD, H, C], BF, kind="ExternalInput")
    d_ob = nc.dram_tensor("ob", [C], F32, kind="ExternalInput")
    d_w1 = nc.dram_tensor("w1", [C, FF], BF, kind="ExternalInput")
    d_b1 = nc.dram_tensor("b1", [FF], F32, kind="ExternalInput")
    d_w2 = nc.dram_tensor("w2", [FF, C], BF, kind="ExternalInput")
    d_bfin = nc.dram_tensor("bfin", [C], F32, kind="ExternalInput")
    d_pw = nc.dram_tensor("pw", [C, C], BF, kind="ExternalInput")
    d_dsag = nc.dram_tensor("dsag", [C], F32, kind="ExternalInput")
    d_dsab = nc.dram_tensor("dsab", [C], F32, kind="ExternalInput")
    d_dw3 = nc.dram_tensor("dw3", [C, 3], F32, kind="ExternalInput")
    d_dsadb = nc.dram_tensor("dsadb", [C], F32, kind="ExternalInput")
    d_ones = nc.dram_tensor("cones", [P, P], F32, kind="ExternalInput")
    d_onesb = nc.dram_tensor("conesb", [P, P], BF, kind="ExternalInput")
    d_invC = nc.dram_tensor("cinvC", [P, 1], BF, kind="ExternalInput")
    d_invCh = nc.dram_tensor("cinvCh", [P, 1], F16, kind="ExternalInput")
    d_mask2 = nc.dram_tensor("mask2", [P, 2], F32, kind="ExternalInput")
    d_out = nc.dram_tensor("outT", [C, TL], F32, kind="ExternalOutput")
    DBG = os.environ.get("BASSDBG", "")
    d_dbg = None
    if DBG:
        d_dbg = nc.dram_tensor("dbgT", [C, T], F32, kind="ExternalOutput")

    eng = nc.vector  # DVE for elementwise

    with tile.TileContext(nc) as tc, nc.allow_low_precision(
            reason="bf16 matmuls validated to 4e-3 rel-l2 against fp32 ref"):
        # ---------- persistent small pools ----------
        consts = tc.alloc_tile_pool(name="consts", bufs=1, side="left")
        ones_r = consts.tile([P, P], F32R, tag="ones")
        nc.sync.dma_start(out=ones_r, in_=d_ones[:, :].bitcast(F32R))
        ones_b = consts.tile([P, P], BF, tag="onesb")
        nc.sync.dma_start(out=ones_b, in_=d_onesb[:, :])
        invC_b = consts.tile([P, 1], BF, tag="invC")
        nc.sync.dma_start(out=invC_b, in_=d_invC[:, :])
        invC_h = consts.tile([P, 1], F16, tag="invCh")
        nc.sync.dma_start(out=invC_h, in_=d_invCh[:, :])
        convb_s = consts.tile([P, CK], F32, tag="convb")
        nc.sync.dma_start(out=convb_s, in_=d_convb.rearrange("(m p) -> p m", p=P))
        bqkv_s = consts.tile([P, 12], F32, tag="bqkv")
        nc.sync.dma_start(out=bqkv_s, in_=d_bqkv.rearrange("(m p) -> p m", p=P))
        ob_s = consts.tile([P, CK], F32, tag="ob")
        nc.sync.dma_start(out=ob_s, in_=d_ob.rearrange("(m p) -> p m", p=P))
        b1_s = consts.tile([P, FFK], F32, tag="b1")
        nc.sync.dma_start(out=b1_s, in_=d_b1.rearrange("(m p) -> p m", p=P))
        bfin_s = consts.tile([P, CK], F32, tag="bfin")
        nc.sync.dma_start(out=bfin_s, in_=d_bfin.rearrange("(m p) -> p m", p=P))
        dsag_s = consts.tile([P, CK], F32, tag="dsag")
        nc.sync.dma_start(out=dsag_s, in_=d_dsag.rearrange("(m p) -> p m", p=P))
        dsab_s = consts.tile([P, CK], F32, tag="dsab")
        nc.sync.dma_start(out=dsab_s, in_=d_dsab.rearrange("(m p) -> p m", p=P))
        dw3_s = consts.tile([P, CK, 3], F32, tag="dw3")
        nc.sync.dma_start(out=dw3_s, in_=d_dw3.rearrange("(m p) d -> p m d", p=P))
        dsadb_s = consts.tile([P, CK], F32, tag="dsadb")
        nc.sync.dma_start(out=dsadb_s, in_=d_dsadb.rearrange("(m p) -> p m", p=P))
        bvbc_s = consts.tile([P, C], F32, tag="bvbc")
        nc.sync.dma_start(out=bvbc_s, in_=d_bvbc[:, :])
        mask2_s = consts.tile([P, 2], F32, tag="mask2")
        nc.sync.dma_start(out=mask2_s, in_=d_mask2[:, :])
        qoff_s = consts.tile([1, 1], U32, tag="qoff")
        nc.sync.dma_start(out=qoff_s, in_=d_qoff[:, :])
        regs = nc.alloc_registers("qoffr")
        nc.regs_load(regs, qoff_s[0:1, 0:1])
        j0 = nc.snap(regs, donate=True, min_val=0, max_val=TL)

        EPS = 1e-5

        # ---------- persistent activation state ----------
        vtmp = tc.alloc_tile_pool(name="vtmp", bufs=2, side="left")
        rmr_pool = tc.alloc_tile_pool(name="rmr", bufs=2, side="left")
        tmp_pool = tc.alloc_tile_pool(name="tmp", bufs=2, side="left")
        hat_pool = tc.alloc_tile_pool(name="hatp", bufs=1, side="left")
        hat = hat_pool.tile([P, CK, T], BF, tag="hat")

        def ln_rmr(psS, src_b, src_h, n_cols):
            """src_b: list of CK [P,n_cols] bf16 APs; src_h: same in f16
            (squares). Returns (r_s, mr_s) fp32 [1,n_cols] vec tiles."""
            ps_mean = psS.tile([1, 512], F32, tag="mean")
            for kc in range(CK):
                nc.tensor.matmul(ps_mean[0:1, :n_cols], invC_b[:, :], src_b[kc],
                                 start=(kc == 0), stop=(kc == CK - 1))
            ps_ex2 = psS.tile([1, 512], F32, tag="ex2")
            for kc in range(CK):
                nc.tensor.matmul(ps_ex2[0:1, :n_cols], invC_h[:, :], src_h[kc],
                                 start=(kc == 0), stop=(kc == CK - 1))
            m_s = vtmp.tile([1, 512], F32, tag="m")
            eng.tensor_copy(out=m_s[:, :n_cols], in_=ps_mean[0:1, :n_cols])
            m2_s = vtmp.tile([1, 512], F32, tag="tmpa")
            eng.tensor_tensor(out=m2_s[:, :n_cols], in0=m_s[:, :n_cols],
                              in1=m_s[:, :n_cols], op=OP.mult)
            # vpe = (ex2 + eps) - m^2
            vpe_s = vtmp.tile([1, 512], F32, tag="tmpa")
            eng.scalar_tensor_tensor(out=vpe_s[:, :n_cols], in0=ps_ex2[0:1, :n_cols],
                                     scalar=EPS, in1=m2_s[:, :n_cols],
                                     op0=OP.add, op1=OP.subtract)
            std_s = vtmp.tile([1, 512], F32, tag="tmpb")
            nc.scalar.activation(out=std_s[:, :n_cols], in_=vpe_s[:, :n_cols],
                                 func=AF.Sqrt)
            r_f = vtmp.tile([1, 512], F32, tag="tmpb")
            eng.reciprocal_approx_fast(out=r_f[:, :n_cols], in_=std_s[:, :n_cols])
            r_s = rmr_pool.tile([1, 512], F32R, tag="r")
            eng.tensor_copy(out=r_s[:, :n_cols], in_=r_f[:, :n_cols])
            mr_s = rmr_pool.tile([1, 512], F32R, tag="mr")
            eng.tensor_tensor(out=mr_s[:, :n_cols], in0=m_s[:, :n_cols],
                              in1=r_f[:, :n_cols], op=OP.mult)
            return r_s, mr_s

        def ln_bcast(psB, r_s, mr_s, n_cols):
            """[P, 2*512] psum tile: [:, :n] = r bcast, [:, 512:512+n] = m*r."""
            ps_bc = psB.tile([P, 1024], F32, tag="lnbc")
            nc.tensor.matmul(ps_bc[:, 0:n_cols], ones_r[0:1, :],
                             r_s[:, :n_cols], start=True, stop=True)
            nc.tensor.matmul(ps_bc[:, 512:512 + n_cols], ones_r[0:1, :],
                             mr_s[:, :n_cols], start=True, stop=True)
            return ps_bc

        # =================== Phase A1: conv + LN1 -> hat ===================
        a1 = tc.alloc_tile_pool(name="a1", bufs=1, side="left")
        convw_s = a1.tile([P, 3, CK, C], BF, tag="convw")
        nc.sync.dma_start(
            out=convw_s,
            in_=d_convw.rearrange("d (k p) o -> p d k o", p=P),
        )
        xch_pool = tc.alloc_tile_pool(name="xch", bufs=3, side="left")
        sq_pool = tc.alloc_tile_pool(name="sq", bufs=1, side="left")
        psA = tc.alloc_tile_pool(name="psA", bufs=2, space="PSUM")
        psS = tc.alloc_tile_pool(name="psS", bufs=1, space="PSUM")
        psB = tc.alloc_tile_pool(name="psB", bufs=2, space="PSUM")

        # conv + gelu + residual into hat (raw), squares into sqs; stats
        # deferred so all ACT gelu/square run before the sqrt table swap.
        sqs = sq_pool.tile([P, CK, T], F16, tag="sqs")
        stats = []
        for n in range(NCH):
            c0 = 512 * n
            x_ch = xch_pool.tile([P, CK, 514], BF, tag="xch")
            nc.sync.dma_start(
                out=x_ch,
                in_=d_xT[:, c0:c0 + 514].rearrange("(k p) t -> p k t", p=P),
            )
            for mo in range(CK):
                ps_c = psA.tile([P, 512], F32, tag="mm")
                first = True
                for dtap in range(3):
                    for kc in range(CK):
                        nc.tensor.matmul(
                            ps_c,
                            convw_s[:, dtap, kc, mo * P:(mo + 1) * P],
                            x_ch[:, kc, dtap:dtap + 512],
                            start=first, stop=(dtap == 2 and kc == CK - 1),
                        )
                        first = False
                f_t = hat[:, mo, c0:c0 + 512]
                nc.scalar.activation(out=f_t, in_=ps_c, func=AF.Gelu,
                                     bias=convb_s[:, mo:mo + 1])
                eng.tensor_tensor(out=f_t, in0=f_t,
                                  in1=x_ch[:, mo, 1:513], op=OP.add)
                eng.tensor_tensor(out=sqs[:, mo, c0:c0 + 512], in0=f_t,
                                  in1=f_t, op=OP.mult)
        for n in range(NCH):
            c0 = 512 * n
            src_b = [hat[:, kc, c0:c0 + 512] for kc in range(CK)]
            src_h = [sqs[:, kc, c0:c0 + 512] for kc in range(CK)]
            r_s, mr_s = ln_rmr(psS, src_b, src_h, 512)
            ps_bc = ln_bcast(psB, r_s, mr_s, 512)
            for kc in range(CK):
                t_s = tmp_pool.tile([P, 512], F32, tag="t")
                eng.tensor_tensor(out=t_s, in0=hat[:, kc, c0:c0 + 512],
                                  in1=ps_bc[:, 0:512], op=OP.mult)
                eng.tensor_tensor(out=hat[:, kc, c0:c0 + 512], in0=t_s,
                                  in1=ps_bc[:, 512:1024], op=OP.subtract)
        if DBG == "hat":
            dbg_pool = tc.alloc_tile_pool(name="dbg", bufs=2, side="left")
            for n in range(NCH):
                c0 = 512 * n
                for kc in range(CK):
                    dbg_t = dbg_pool.tile([P, 512], F32, tag="dbg")
                    eng.tensor_copy(out=dbg_t, in_=hat[:, kc, c0:c0 + 512])
                    nc.sync.dma_start(out=d_dbg[kc * P:(kc + 1) * P, c0:c0 + 512],
                                      in_=dbg_t)
            dbg_pool.release()
        for pool in (sq_pool, xch_pool, a1):
            pool.release()

        # =================== Phase A2: K, V, Q ===================
        kv_state = tc.alloc_tile_pool(name="kvst", bufs=1, side="right")
        st_pool = tc.alloc_tile_pool(name="stage", bufs=2, side="right")
        a2 = tc.alloc_tile_pool(name="a2", bufs=1, side="right")
        wkv_s = a2.tile([P, CK, 2 * C], BF, tag="wkv")
        nc.sync.dma_start(
            out=wkv_s,
            in_=d_wqkv.rearrange("(k p) o -> p k o", p=P)[:, :, C:3 * C],
        )
        kaug = kv_state.tile([HD + 1, H, T], BF, tag="kaug")
        qaug = kv_state.tile([HD + 1, H, TL], BF, tag="qaug")
        vsb = kv_state.tile([P, TK, H, HD + 1], BF, tag="v")

        # v ones column
        eng.tensor_copy(out=vsb[:, :, :, HD],
                        in_=ones_b.rearrange("p (g h) -> p g h", h=H)[:, 0:TK, :])
        # aug rows
        for h in range(H):
            nc.sync.dma_start(out=kaug[HD:HD + 1, h, :], in_=d_A[:, :])
            nc.sync.dma_start(out=qaug[HD:HD + 1, h, :], in_=d_qA[:, :])

        for n in range(NCH):
            c0 = 512 * n
            # K tiles
            for mo in range(CK):
                ps_k = psA.tile([P, 512], F32, tag="mm")
                for kc in range(CK):
                    nc.tensor.matmul(ps_k, wkv_s[:, kc, mo * P:(mo + 1) * P],
                                     hat[:, kc, c0:c0 + 512],
                                     start=(kc == 0), stop=(kc == CK - 1))
                st = st_pool.tile([P, 512], BF, tag="kst")
                eng.tensor_scalar(out=st, in0=ps_k, scalar1=bqkv_s[:, 4 + mo:5 + mo],
                                  scalar2=None, op0=OP.add)
                nc.sync.dma_start(out=kaug[0:HD, 2 * mo, c0:c0 + 512], in_=st[0:HD, :])
                nc.sync.dma_start(out=kaug[0:HD, 2 * mo + 1, c0:c0 + 512], in_=st[HD:P, :])
            # V tiles (natural layout)
            for tt in range(4):
                g = 4 * n + tt
                ps_v = psA.tile([P, 512], F32, tag="mm")
                for kc in range(CK):
                    nc.tensor.matmul(ps_v, hat[:, kc, c0 + tt * P:c0 + (tt + 1) * P],
                                     wkv_s[:, kc, C:2 * C],
                                     start=(kc == 0), stop=(kc == CK - 1))
                eng.tensor_tensor(out=vsb[:, g, :, 0:HD],
                                  in0=ps_v.rearrange("p (h d) -> p h d", d=HD),
                                  in1=bvbc_s.rearrange("p (h d) -> p h d", d=HD),
                                  op=OP.add)
        # Q tiles (local half via dynamic offset)
        a2.release()
        a2q = tc.alloc_tile_pool(name="a2q", bufs=1, side="right")
        wq_s = a2q.tile([P, CK, C], BF, tag="wq")
        nc.sync.dma_start(
            out=wq_s,
            in_=d_wqkv.rearrange("(k p) o -> p k o", p=P)[:, :, 0:C],
        )
        for mo in range(CK):
            for n2 in range(NL):
                ps_q = psA.tile([P, 512], F32, tag="mm")
                for kc in range(CK):
                    nc.tensor.matmul(ps_q, wq_s[:, kc, mo * P:(mo + 1) * P],
                                     hat[:, kc, bass.ds(j0 + n2 * 512, 512)],
                                     start=(kc == 0), stop=(kc == CK - 1))
                st = st_pool.tile([P, 512], BF, tag="kst")
                eng.tensor_scalar(out=st, in0=ps_q, scalar1=bqkv_s[:, mo:mo + 1],
                                  scalar2=None, op0=OP.add)
                nc.sync.dma_start(out=qaug[0:HD, 2 * mo, n2 * 512:(n2 + 1) * 512],
                                  in_=st[0:HD, :])
                nc.sync.dma_start(out=qaug[0:HD, 2 * mo + 1, n2 * 512:(n2 + 1) * 512],
                                  in_=st[HD:P, :])
        for pool in (a2q, st_pool, hat_pool, psB, psS, psA):
            pool.release()

        # =================== Attention ===================
        attn_state = tc.alloc_tile_pool(name="attnst", bufs=1, side="left")
        attnh = attn_state.tile([HD, H, TL], BF, tag="attnh")
        p_pool = tc.alloc_tile_pool(name="pp", bufs=2, side="right")
        psS2 = tc.alloc_tile_pool(name="psS2", bufs=2, space="PSUM")
        psAV = tc.alloc_tile_pool(name="psAV", bufs=2, space="PSUM")

        for h in range(H):
            ps_av = psAV.tile([HD + 1, 1024], F32, tag="av")
            for tk in range(TK):
                ps_s = psS2.tile([P, 1024], F32, tag="score")
                for n2 in range(NL):
                    nc.tensor.matmul(ps_s[:, n2 * 512:(n2 + 1) * 512],
                                     kaug[:, h, tk * P:(tk + 1) * P],
                                     qaug[:, h, n2 * 512:(n2 + 1) * 512],
                                     start=True, stop=True)
                p_t = p_pool.tile([P, 1024], BF, tag="p")
                nc.scalar.activation(out=p_t, in_=ps_s, func=AF.Exp)
                for n2 in range(NL):
                    nc.tensor.matmul(ps_av[:, n2 * 512:(n2 + 1) * 512],
                                     vsb[:, tk, h, :],
                                     p_t[:, n2 * 512:(n2 + 1) * 512],
                                     start=(tk == 0), stop=(tk == TK - 1))
            for n2 in range(NL):
                cc = slice(n2 * 512, (n2 + 1) * 512)
                if DBG == "den2":
                    dd = vtmp.tile([1, 512], F32, tag="dbgden")
                    eng.tensor_copy(out=dd, in_=ps_av[HD:HD + 1, cc])
                    nc.sync.dma_start(out=d_dbg[16 + h:17 + h, cc], in_=dd)
                    da = vtmp.tile([1, 512], F32, tag="dbgav")
                    eng.tensor_copy(out=da, in_=ps_av[0:1, cc])
                    nc.sync.dma_start(out=d_dbg[24 + h:25 + h, cc], in_=da)

                den_s = vtmp.tile([1, 512], F32, tag="tmpb")
                eng.tensor_copy(out=den_s, in_=ps_av[HD:HD + 1, cc])
                d_f = vtmp.tile([1, 512], F32, tag="tmpa")
                eng.reciprocal_approx_fast(out=d_f, in_=den_s)
                d_s = vtmp.tile([1, 512], F32R, tag="d")
                eng.tensor_copy(out=d_s, in_=d_f)
                ps_b = psS2.tile([P, 1024], F32, tag="score")
                nc.tensor.matmul(ps_b[0:HD, 0:512], ones_r[0:1, 0:HD],
                                 d_s, start=True, stop=True)
                db_s = tmp_pool.tile([HD, 512], F32, tag="dbs")
                eng.tensor_copy(out=db_s, in_=ps_b[0:HD, 0:512])
                if DBG == "den2":
                    nc.sync.dma_start(out=d_dbg[40 + h:41 + h, cc], in_=db_s[0:1, :])
                eng.tensor_tensor(out=attnh[:, h, cc], in0=ps_av[0:HD, cc],
                                  in1=db_s, op=OP.mult)
        if DBG == "den2":
            dbg_pool4 = tc.alloc_tile_pool(name="dbg4", bufs=2, side="right")
            for h in range(4):
                t5 = dbg_pool4.tile([HD, 512], F32, tag="q")
                eng.tensor_copy(out=t5, in_=qaug[0:HD, h, 0:512])
                nc.sync.dma_start(out=d_dbg[256 + h * 64:256 + (h + 1) * 64, 0:512],
                                  in_=t5)
            dbg_pool4.release()
        if DBG == "den":
            dbg_pool3 = tc.alloc_tile_pool(name="dbg3", bufs=2, side="right")
            for h in range(H):
                t1 = dbg_pool3.tile([1, T], F32, tag="a")
                eng.tensor_copy(out=t1, in_=kaug[HD:HD + 1, h, :])
                nc.sync.dma_start(out=d_dbg[h:h + 1, 0:T], in_=t1)
                t2 = dbg_pool3.tile([1, TL], F32, tag="b")
                eng.tensor_copy(out=t2, in_=qaug[HD:HD + 1, h, :])
                nc.sync.dma_start(out=d_dbg[8 + h:9 + h, 0:TL], in_=t2)
                t3 = dbg_pool3.tile([P, TK], F32, tag="c")
                eng.tensor_copy(out=t3, in_=vsb[:, :, h, HD])
                nc.sync.dma_start(out=d_dbg[128:256, h * TK:(h + 1) * TK], in_=t3)
                t4 = dbg_pool3.tile([HD, 512], F32, tag="d")
                eng.tensor_copy(out=t4, in_=kaug[0:HD, h, 0:512])
                nc.sync.dma_start(out=d_dbg[256 + h * 16:256 + h * 16 + HD // 4, 0:512],
                                  in_=t4[0:HD // 4, :])
            dbg_pool3.release()
        if DBG == "attnh":
            dbg_pool2 = tc.alloc_tile_pool(name="dbg2", bufs=2, side="right")
            for h in range(H):
                for n2 in range(NL):
                    cc = slice(n2 * 512, (n2 + 1) * 512)
                    dbg_t = dbg_pool2.tile([HD, 512], F32, tag="dbg")
                    eng.tensor_copy(out=dbg_t, in_=attnh[:, h, cc])
                    nc.sync.dma_start(out=d_dbg[h * HD:(h + 1) * HD, cc], in_=dbg_t)
            dbg_pool2.release()
        for pool in (p_pool, kv_state, psAV, psS2):
            pool.release()

        # =================== out-proj + residual + LN2 ===================
        late = tc.alloc_tile_pool(name="late", bufs=1, side="right")
        wo_s = late.tile([HD, H, C], BF, tag="wo")
        nc.sync.dma_start(out=wo_s, in_=d_wo[:, :, :])
        xd_s = late.tile([P, CK, TL + 2], F32, tag="xd")
        nc.sync.dma_start(out=xd_s,
                          in_=d_xd.rearrange("(k p) t -> p k t", p=P))
        xdb_s = late.tile([P, CK, TL + 2], BF, tag="xdb")
        nc.sync.dma_start(out=xdb_s,
                          in_=d_xdb.rearrange("(k p) t -> p k t", p=P))
        ftc2 = late.tile([P, CK, TL], BF, tag="ftc2")
        sq2 = late.tile([P, CK, TL], F16, tag="sq2")
        w1_s = late.tile([P, CK, FF], BF, tag="w1")
        nc.sync.dma_start(out=w1_s,
                          in_=d_w1.rearrange("(k p) o -> p k o", p=P))
        w2_s = late.tile([P, FFK, C], BF, tag="w2")
        nc.sync.dma_start(out=w2_s,
                          in_=d_w2.rearrange("(k p) o -> p k o", p=P))
        pw_s = late.tile([P, CK, C], BF, tag="pw")
        dsa_out = late.tile([P, CK, TL], F32, tag="dsaout")
        nc.sync.dma_start(out=pw_s,
                          in_=d_pw.rearrange("(k p) o -> p k o", p=P))

        psC = tc.alloc_tile_pool(name="psC", bufs=2, space="PSUM")
        psS_l = tc.alloc_tile_pool(name="psSl", bufs=1, space="PSUM")
        psB_l = tc.alloc_tile_pool(name="psBl", bufs=1, space="PSUM")
        for mo in range(CK):
            for n2 in range(NL):
                cc = slice(n2 * 512, (n2 + 1) * 512)
                ps_o = psC.tile([P, 512], F32, tag="mm")
                for h in range(H):
                    nc.tensor.matmul(ps_o, wo_s[:, h, mo * P:(mo + 1) * P],
                                     attnh[:, h, cc],
                                     start=(h == 0), stop=(h == H - 1))
                eng.scalar_tensor_tensor(
                    out=ftc2[:, mo, cc], in0=ps_o, scalar=ob_s[:, mo:mo + 1],
                    in1=xd_s[:, mo, 1 + n2 * 512:1 + (n2 + 1) * 512],
                    op0=OP.add, op1=OP.add)
                eng.tensor_tensor(out=sq2[:, mo, cc], in0=ftc2[:, mo, cc],
                                  in1=ftc2[:, mo, cc], op=OP.mult)
        attn_state.release()

        # =================== DSA branch (LN + stats share sqrt table) =====
        dsa_pool = tc.alloc_tile_pool(name="dsap", bufs=1, side="right")
        z_s = dsa_pool.tile([P, CK, TL + 2], BF, tag="z")
        z1_s = dsa_pool.tile([P, CK, TL], BF, tag="z1")
        sqd = dsa_pool.tile([P, CK, TL + 2], F16, tag="sqd")
        for kc in range(CK):
            eng.tensor_tensor(out=sqd[:, kc, :], in0=xdb_s[:, kc, :],
                              in1=xdb_s[:, kc, :], op=OP.mult)

        # LN2 stats+apply (2 groups), then DSA LN (3 groups) — all Sqrt sites
        # stay contiguous so the ACT table swaps only twice around them.
        for n2 in range(NL):
            cc = slice(n2 * 512, (n2 + 1) * 512)
            src_b = [ftc2[:, kc, cc] for kc in range(CK)]
            src_h = [sq2[:, kc, cc] for kc in range(CK)]
            r_s, mr_s = ln_rmr(psS_l, src_b, src_h, 512)
            ps_bc = ln_bcast(psB_l, r_s, mr_s, 512)
            for kc in range(CK):
                t_s = tmp_pool.tile([P, 512], F32, tag="t")
                eng.tensor_tensor(out=t_s, in0=ftc2[:, kc, cc],
                                  in1=ps_bc[:, 0:512], op=OP.mult)
                eng.tensor_tensor(out=ftc2[:, kc, cc], in0=t_s,
                                  in1=ps_bc[:, 512:1024], op=OP.subtract)
        # DSA LN apply -> z (gamma/beta via tensor_scalar)
        for (c0, w) in ((0, 512), (512, 512), (1024, 2)):
            src_b = [xdb_s[:, kc, c0:c0 + w] for kc in range(CK)]
            src_h = [sqd[:, kc, c0:c0 + w] for kc in range(CK)]
            r_s, mr_s = ln_rmr(psS_l, src_b, src_h, w)
            ps_bc = ln_bcast(psB_l, r_s, mr_s, w)
            for kc in range(CK):
                t_s = tmp_pool.tile([P, 512], F32, tag="t")
                eng.tensor_tensor(out=t_s[:, :w], in0=xd_s[:, kc, c0:c0 + w],
                                  in1=ps_bc[:, 0:w], op=OP.mult)
                eng.tensor_tensor(out=t_s[:, :w], in0=t_s[:, :w],
                                  in1=ps_bc[:, 512:512 + w], op=OP.subtract)
                eng.tensor_scalar(out=z_s[:, kc, c0:c0 + w], in0=t_s[:, :w],
                                  scalar1=dsag_s[:, kc:kc + 1], scalar2=dsab_s[:, kc:kc + 1],
                                  op0=OP.mult, op1=OP.add)
        # gate the two halo columns: 1 for a valid neighbor column, 0 at the
        # true batch boundary (per-core mask2 input)
        for i, cpad in enumerate((0, TL + 1)):
            eng.tensor_scalar(out=z_s[:, :, cpad:cpad + 1],
                              in0=z_s[:, :, cpad:cpad + 1],
                              scalar1=mask2_s[:, i:i + 1], scalar2=None, op0=OP.mult)
        for pool in (psB_l, psS_l):
            pool.release()
        for kc in range(CK):
            eng.tensor_scalar(out=z1_s[:, kc, :], in0=z_s[:, kc, 0:TL],
                              scalar1=dw3_s[:, kc, 0:1], scalar2=None, op0=OP.mult)
            eng.scalar_tensor_tensor(out=z1_s[:, kc, :], in0=z_s[:, kc, 1:1 + TL],
                                     scalar=dw3_s[:, kc, 1:2],
                                     in1=z1_s[:, kc, :],
                                     op0=OP.mult, op1=OP.add)
            eng.scalar_tensor_tensor(out=z1_s[:, kc, :], in0=z_s[:, kc, 2:2 + TL],
                                     scalar=dw3_s[:, kc, 2:3],
                                     in1=z1_s[:, kc, :],
                                     op0=OP.mult, op1=OP.add)
            nc.scalar.activation(out=z1_s[:, kc, :], in_=z1_s[:, kc, :],
                                 func=AF.Gelu, bias=dsadb_s[:, kc:kc + 1])
        for mo in range(CK):
            for n2 in range(NL):
                cc = slice(n2 * 512, (n2 + 1) * 512)
                ps_d = psC.tile([P, 512], F32, tag="mm")
                for kc in range(CK):
                    nc.tensor.matmul(ps_d, pw_s[:, kc, mo * P:(mo + 1) * P],
                                     z1_s[:, kc, cc],
                                     start=(kc == 0), stop=(kc == CK - 1))
                eng.tensor_copy(out=dsa_out[:, mo, cc], in_=ps_d)
        dsa_pool.release()

        # =================== MLP + final combine ===================
        hh_pool = tc.alloc_tile_pool(name="hh", bufs=3, side="left")
        fin_pool = tc.alloc_tile_pool(name="fin", bufs=3, side="left")
        psO = tc.alloc_tile_pool(name="psO", bufs=1, space="PSUM")
        for n2 in range(NL):
            cc = slice(n2 * 512, (n2 + 1) * 512)
            ps_out = [psO.tile([P, 512], F32, tag=f"out{mo}", name=f"psout{mo}") for mo in range(CK)]
            for ff in range(FFK):
                ps_h = psC.tile([P, 512], F32, tag="mm")
                for kc in range(CK):
                    nc.tensor.matmul(ps_h, w1_s[:, kc, ff * P:(ff + 1) * P],
                                     ftc2[:, kc, cc],
                                     start=(kc == 0), stop=(kc == CK - 1))
                hh_t = hh_pool.tile([P, 512], BF, tag="hh")
                nc.scalar.activation(out=hh_t, in_=ps_h, func=AF.Gelu,
                                     bias=b1_s[:, ff:ff + 1])
                for mo in range(CK):
                    nc.tensor.matmul(ps_out[mo], w2_s[:, ff, mo * P:(mo + 1) * P],
                                     hh_t, start=(ff == 0), stop=(ff == FFK - 1))
            for mo in range(CK):
                fin_t = fin_pool.tile([P, 512], F32, tag="fin")
                eng.scalar_tensor_tensor(out=fin_t, in0=ps_out[mo],
                                         scalar=bfin_s[:, mo:mo + 1],
                                         in1=dsa_out[:, mo, cc],
                                         op0=OP.add, op1=OP.add)
                nc.sync.dma_start(out=d_out[mo * P:(mo + 1) * P, cc], in_=fin_t)

        for pool in (fin_pool, hh_pool, late, tmp_pool, rmr_pool, vtmp,
                     consts, psO, psC):
            pool.release()

    nc.compile()
    return nc


def _in_maps(inputs):
    f = lambda v: np.ascontiguousarray(np.asarray(v), dtype=np.float32)
    bf = lambda v: np.ascontiguousarray(np.asarray(v, dtype=np.float32).astype(ml_dtypes.bfloat16))
    x = f(inputs["x"])            # [B, T, C]
    A = f(inputs["A"])            # [B, T]
    alpha = float(np.asarray(inputs["alpha_bias"]).reshape(-1)[0])
    dst_a = float(np.asarray(inputs["dst_alpha"]))
    dst_b = float(np.asarray(inputs["dst_beta"]))
    conv1_w, conv1_b = f(inputs["conv1_w"]), f(inputs["conv1_b"])
    ln1_g, ln1_b = f(inputs["ln1_g"]), f(inputs["ln1_b"])
    in_w, in_b = f(inputs["in_proj_w"]), f(inputs["in_proj_b"])
    out_w, out_b = f(inputs["out_w"]), f(inputs["out_b"])
    ln2_g, ln2_b = f(inputs["ln2_g"]), f(inputs["ln2_b"])
    w1, b1 = f(inputs["mlp_w1"]), f(inputs["mlp_b1"])
    w2, b2 = f(inputs["mlp_w2"]), f(inputs["mlp_b2"])
    dsa_g, dsa_b = f(inputs["dsa_ln_g"]), f(inputs["dsa_ln_b"])
    dsa_dw, dsa_db = f(inputs["dsa_dw"]), f(inputs["dsa_db"])
    dsa_pw, dsa_pb = f(inputs["dsa_pw"]), f(inputs["dsa_pb"])

    weff = in_w * ln1_g[None, :]
    beff = in_w @ ln1_b + in_b
    weff[:C] /= np.sqrt(HD).astype(np.float32)
    beff[:C] /= np.sqrt(HD).astype(np.float32)
    shared = {
        "convw": bf(np.transpose(conv1_w, (2, 1, 0))),
        "convb": conv1_b,
        "wqkv": bf(weff.T),
        "bqkv": beff,
        "bvbc": np.ascontiguousarray(np.broadcast_to(beff[2 * C:3 * C], (P, C))),
        "wo": bf(out_w.T.reshape(HD * H, C).reshape(H, HD, C).transpose(1, 0, 2)),
        "ob": out_b,
        "w1": bf((w1 * ln2_g[None, :]).T),
        "b1": w1 @ ln2_b + b1,
        "w2": bf((dst_a * w2).T),
        "bfin": dst_a * b2 + dst_b * dsa_pb,
        "pw": bf((dst_b * dsa_pw[:, :, 0]).T),
        "dsag": dsa_g, "dsab": dsa_b,
        "dw3": dsa_dw[:, 0, :], "dsadb": dsa_db,
        "cones": np.ones((P, P), np.float32),
        "conesb": np.ones((P, P), ml_dtypes.bfloat16),
        "cinvC": np.full((P, 1), 1.0 / C, ml_dtypes.bfloat16),
        "cinvCh": np.full((P, 1), 1.0 / C, np.float16),
    }
    maps = []
    for core in range(8):
        b, half = core // 2, core % 2
        j0 = half * TL
        xT = np.zeros((C, T + 2), np.float32)
        xT[:, 1:T + 1] = x[b].T
        xd = np.zeros((C, TL + 2), np.float32)
        lo, hi = j0 - 1, j0 + TL + 1
        slo, shi = max(lo, 0), min(hi, T)
        xd[:, slo - lo:slo - lo + (shi - slo)] = x[b].T[:, slo:shi]
        m = dict(shared)
        m["xT"] = bf(xT)
        m["xd"] = xd
        m["xdb"] = bf(xd)
        mask2 = np.ones((P, 2), np.float32)
        if lo < 0:
            mask2[:, 0] = 0.0
        if hi > T:
            mask2[:, 1] = 0.0
        m["mask2"] = mask2
        m["Arow"] = bf(A[b:b + 1, :])
        m["qArow"] = bf(alpha * A[b:b + 1, j0:j0 + TL])
        m["qoff"] = np.array([[j0]], np.uint32)
        maps.append(m)
    return maps


def _get_program():
    global _CACHED
    if _CACHED is None:
        _CACHED = _build()
    return _CACHED


def kernel(**inputs):
    nc = _get_program()
    maps = _in_maps(inputs)
    res = run_bass_kernel_spmd(nc, maps, list(range(8)))
    out = np.empty((B, T, C), np.float32)
    for core in range(8):
        b, half = core // 2, core % 2
        out[b, half * TL:(half + 1) * TL, :] = res.results[core]["outT"].T
    return out


# revision 34
# speedup vs baseline: 1.3119x; 1.1442x over previous
"""nn_BoundaryGuidedDSTLayer Trainium2 Bass kernel (8-core SPMD, no collectives).

Sharding: core c = (b = c//2, half = c%2). Each core computes the conv
pre-mix + LN1 + K/V over the full T of its batch, and Q / attention /
out-proj / MLP / DSA for its local 1024-column half. Activations are
transposed [C, T]; all big matmuls run in bf16 (fp32 PSUM).

v3: software-pipelined attention (scores of head h run concurrently
with AV of head h-1 via an SBUF p buffer), deeper PSUM buffering on
the GEMM phases, DSA/LN2/MLP reordered so DVE chains overlap PE.
"""
import sys, os

for _p in ("/opt/trn_rl_repo",):
    if os.path.isdir(_p) and _p not in sys.path:
        sys.path.append(_p)

import numpy as np
import ml_dtypes
import concourse.bass as bass
import concourse.mybir as mybir
import concourse.tile as tile
from concourse.bacc import Bacc
from concourse.bass_utils import run_bass_kernel_spmd

dt = mybir.dt
F32, F32R, U32 = dt.float32, dt.float32r, dt.uint32
BF, F16 = dt.bfloat16, dt.float16
AF = mybir.ActivationFunctionType
OP = mybir.AluOpType

P = 128
B, T, C, H = 4, 2048, 512, 8
HD = C // H          # 64
FF = 4 * C           # 2048
TL = T // 2          # 1024 local columns per core
CK = C // P          # 4
FFK = FF // P        # 16
NCH = T // 512       # 4 chunks over full T
NL = TL // 512       # 2 chunks over local T
TK = T // P          # 16 key tiles

_CACHED = None


def _build():
    nc = Bacc("TRN2", target_bir_lowering=False, debug=False, num_devices=8)

    # ---- DRAM I/O ----
    d_xT = nc.dram_tensor("xT", [C, T + 2], BF, kind="ExternalInput")
    d_xd = nc.dram_tensor("xd", [C, TL + 2], F32, kind="ExternalInput")
    d_xdb = nc.dram_tensor("xdb", [C, TL + 2], BF, kind="ExternalInput")
    d_A = nc.dram_tensor("Arow", [1, T], BF, kind="ExternalInput")
    d_qA = nc.dram_tensor("qArow", [1, TL], BF, kind="ExternalInput")
    d_qoff = nc.dram_tensor("qoff", [1, 1], U32, kind="ExternalInput")
    d_convw = nc.dram_tensor("convw", [3, C, C], BF, kind="ExternalInput")
    d_convb = nc.dram_tensor("convb", [C], F32, kind="ExternalInput")
    d_wqkv = nc.dram_tensor("wqkv", [C, 3 * C], BF, kind="ExternalInput")
    d_bqkv = nc.dram_tensor("bqkv", [3 * C], F32, kind="ExternalInput")
    d_bvbc = nc.dram_tensor("bvbc", [P, C], F32, kind="ExternalInput")
    d_wo = nc.dram_tensor("wo", [HD, H, C], BF, kind="ExternalInput")
    d_ob = nc.dram_tensor("ob", [C], F32, kind="ExternalInput")
    d_w1 = nc.dram_tensor("w1", [C, FF], BF, kind="ExternalInput")
    d_b1 = nc.dram_tensor("b1", [FF], F32, kind="ExternalInput")
    d_w2 = nc.dram_tensor("w2", [FF, C], BF, kind="ExternalInput")
    d_bfin = nc.dram_tensor("bfin", [C], F32, kind="ExternalInput")
    d_pw = nc.dram_tensor("pw", [C, C], BF, kind="ExternalInput")
    d_dsag = nc.dram_tensor("dsag", [C], F32, kind="ExternalInput")
    d_dsab = nc.dram_tensor("dsab", [C], F32, kind="ExternalInput")
    d_dw3 = nc.dram_tensor("dw3", [C, 3], F32, kind="ExternalInput")
    d_dsadb = nc.dram_tensor("dsadb", [C], F32, kind="ExternalInput")
    d_ones = nc.dram_tensor("cones", [P, P], F32, kind="ExternalInput")
    d_onesb = nc.dram_tensor("conesb", [P, P], BF, kind="ExternalInput")
    d_invC = nc.dram_tensor("cinvC", [P, 1], BF, kind="ExternalInput")
    d_invCh = nc.dram_tensor("cinvCh", [P, 1], F16, kind="ExternalInput")
    d_mask2 = nc.dram_tensor("mask2", [P, 2], F32, kind="ExternalInput")
    d_out = nc.dram_tensor("outT", [C, TL], F32, kind="ExternalOutput")

    eng = nc.vector  # DVE for elementwise

    with tile.TileContext(nc) as tc, nc.allow_low_precision(
            reason="bf16 matmuls validated to 4e-3 rel-l2 against fp32 ref"):
        # ---------- persistent small pools ----------
        consts = tc.alloc_tile_pool(name="consts", bufs=1, side="left")
        ones_r = consts.tile([P, P], F32R, tag="ones")
        nc.sync.dma_start(out=ones_r, in_=d_ones[:, :].bitcast(F32R))
        ones_b = consts.tile([P, P], BF, tag="onesb")
        nc.sync.dma_start(out=ones_b, in_=d_onesb[:, :])
        invC_b = consts.tile([P, 1], BF, tag="invC")
        nc.sync.dma_start(out=invC_b, in_=d_invC[:, :])
        invC_h = consts.tile([P, 1], F16, tag="invCh")
        nc.sync.dma_start(out=invC_h, in_=d_invCh[:, :])
        convb_s = consts.tile([P, CK], F32, tag="convb")
        nc.sync.dma_start(out=convb_s, in_=d_convb.rearrange("(m p) -> p m", p=P))
        bqkv_s = consts.tile([P, 12], F32, tag="bqkv")
        nc.sync.dma_start(out=bqkv_s, in_=d_bqkv.rearrange("(m p) -> p m", p=P))
        ob_s = consts.tile([P, CK], F32, tag="ob")
        nc.sync.dma_start(out=ob_s, in_=d_ob.rearrange("(m p) -> p m", p=P))
        b1_s = consts.tile([P, FFK], F32, tag="b1")
        nc.sync.dma_start(out=b1_s, in_=d_b1.rearrange("(m p) -> p m", p=P))
        bfin_s = consts.tile([P, CK], F32, tag="bfin")
        nc.sync.dma_start(out=bfin_s, in_=d_bfin.rearrange("(m p) -> p m", p=P))
        dsag_s = consts.tile([P, CK], F32, tag="dsag")
        nc.sync.dma_start(out=dsag_s, in_=d_dsag.rearrange("(m p) -> p m", p=P))
        dsab_s = consts.tile([P, CK], F32, tag="dsab")
        nc.sync.dma_start(out=dsab_s, in_=d_dsab.rearrange("(m p) -> p m", p=P))
        dw3_s = consts.tile([P, CK, 3], F32, tag="dw3")
        nc.sync.dma_start(out=dw3_s, in_=d_dw3.rearrange("(m p) d -> p m d", p=P))
        dsadb_s = consts.tile([P, CK], F32, tag="dsadb")
        nc.sync.dma_start(out=dsadb_s, in_=d_dsadb.rearrange("(m p) -> p m", p=P))
        bvbc_s = consts.tile([P, C], F32, tag="bvbc")
        nc.sync.dma_start(out=bvbc_s, in_=d_bvbc[:, :])
        mask2_s = consts.tile([P, 2], F32, tag="mask2")
        nc.sync.dma_start(out=mask2_s, in_=d_mask2[:, :])
        qoff_s = consts.tile([1, 1], U32, tag="qoff")
        nc.sync.dma_start(out=qoff_s, in_=d_qoff[:, :])
        regs = nc.alloc_registers("qoffr")
        nc.regs_load(regs, qoff_s[0:1, 0:1])
        j0 = nc.snap(regs, donate=True, min_val=0, max_val=TL)

        EPS = 1e-5

        vtmp = tc.alloc_tile_pool(name="vtmp", bufs=2, side="left")
        rmr_pool = tc.alloc_tile_pool(name="rmr", bufs=2, side="left")
        tmp_pool = tc.alloc_tile_pool(name="tmp", bufs=2, side="left")
        hat_pool = tc.alloc_tile_pool(name="hatp", bufs=1, side="left")
        hat = hat_pool.tile([P, CK, T], BF, tag="hat")

        def ln_rmr(psS, src_b, src_h, n_cols):
            ps_mean = psS.tile([1, 512], F32, tag="mean")
            for kc in range(CK):
                nc.tensor.matmul(ps_mean[0:1, :n_cols], invC_b[:, :], src_b[kc],
                                 start=(kc == 0), stop=(kc == CK - 1))
            ps_ex2 = psS.tile([1, 512], F32, tag="ex2")
            for kc in range(CK):
                nc.tensor.matmul(ps_ex2[0:1, :n_cols], invC_h[:, :], src_h[kc],
                                 start=(kc == 0), stop=(kc == CK - 1))
            m_s = vtmp.tile([1, 512], F32, tag="m")
            eng.tensor_copy(out=m_s[:, :n_cols], in_=ps_mean[0:1, :n_cols])
            m2_s = vtmp.tile([1, 512], F32, tag="tmpa")
            eng.tensor_tensor(out=m2_s[:, :n_cols], in0=m_s[:, :n_cols],
                              in1=m_s[:, :n_cols], op=OP.mult)
            vpe_s = vtmp.tile([1, 512], F32, tag="tmpa")
            eng.scalar_tensor_tensor(out=vpe_s[:, :n_cols], in0=ps_ex2[0:1, :n_cols],
                                     scalar=EPS, in1=m2_s[:, :n_cols],
                                     op0=OP.add, op1=OP.subtract)
            std_s = vtmp.tile([1, 512], F32, tag="tmpb")
            nc.scalar.activation(out=std_s[:, :n_cols], in_=vpe_s[:, :n_cols],
                                 func=AF.Sqrt)
            r_f = vtmp.tile([1, 512], F32, tag="tmpb")
            eng.reciprocal_approx_fast(out=r_f[:, :n_cols], in_=std_s[:, :n_cols])
            r_s = rmr_pool.tile([1, 512], F32R, tag="r")
            eng.tensor_copy(out=r_s[:, :n_cols], in_=r_f[:, :n_cols])
            mr_s = rmr_pool.tile([1, 512], F32R, tag="mr")
            eng.tensor_tensor(out=mr_s[:, :n_cols], in0=m_s[:, :n_cols],
                              in1=r_f[:, :n_cols], op=OP.mult)
            return r_s, mr_s

        def ln_bcast(psB, r_s, mr_s, n_cols):
            ps_bc = psB.tile([P, 1024], F32, tag="lnbc")
            nc.tensor.matmul(ps_bc[:, 0:n_cols], ones_r[0:1, :],
                             r_s[:, :n_cols], start=True, stop=True)
            nc.tensor.matmul(ps_bc[:, 512:512 + n_cols], ones_r[0:1, :],
                             mr_s[:, :n_cols], start=True, stop=True)
            return ps_bc

        # =================== Phase A1: conv + LN1 -> hat ===================
        a1 = tc.alloc_tile_pool(name="a1", bufs=1, side="left")
        convw_s = a1.tile([P, 3, CK, C], BF, tag="convw")
        nc.sync.dma_start(
            out=convw_s,
            in_=d_convw.rearrange("d (k p) o -> p d k o", p=P),
        )
        xch_pool = tc.alloc_tile_pool(name="xch", bufs=3, side="left")
        sq_pool = tc.alloc_tile_pool(name="sq", bufs=1, side="left")
        psA = tc.alloc_tile_pool(name="psA", bufs=4, space="PSUM")
        psS = tc.alloc_tile_pool(name="psS", bufs=1, space="PSUM")
        psB = tc.alloc_tile_pool(name="psB", bufs=1, space="PSUM")

        sqs = sq_pool.tile([P, CK, T], F16, tag="sqs")
        for n in range(NCH):
            c0 = 512 * n
            x_ch = xch_pool.tile([P, CK, 514], BF, tag="xch")
            nc.sync.dma_start(
                out=x_ch,
                in_=d_xT[:, c0:c0 + 514].rearrange("(k p) t -> p k t", p=P),
            )
            for mo in range(CK):
                ps_c = psA.tile([P, 512], F32, tag="mm")
                first = True
                for dtap in range(3):
                    for kc in range(CK):
                        nc.tensor.matmul(
                            ps_c,
                            convw_s[:, dtap, kc, mo * P:(mo + 1) * P],
                            x_ch[:, kc, dtap:dtap + 512],
                            start=first, stop=(dtap == 2 and kc == CK - 1),
                        )
                        first = False
                f_t = hat[:, mo, c0:c0 + 512]
                nc.scalar.activation(out=f_t, in_=ps_c, func=AF.Gelu,
                                     bias=convb_s[:, mo:mo + 1])
                eng.tensor_tensor(out=f_t, in0=f_t,
                                  in1=x_ch[:, mo, 1:513], op=OP.add)
                eng.tensor_tensor(out=sqs[:, mo, c0:c0 + 512], in0=f_t,
                                  in1=f_t, op=OP.mult)
        for n in range(NCH):
            c0 = 512 * n
            src_b = [hat[:, kc, c0:c0 + 512] for kc in range(CK)]
            src_h = [sqs[:, kc, c0:c0 + 512] for kc in range(CK)]
            r_s, mr_s = ln_rmr(psS, src_b, src_h, 512)
            ps_bc = ln_bcast(psB, r_s, mr_s, 512)
            for kc in range(CK):
                t_s = tmp_pool.tile([P, 512], F32, tag="t")
                eng.tensor_tensor(out=t_s, in0=hat[:, kc, c0:c0 + 512],
                                  in1=ps_bc[:, 0:512], op=OP.mult)
                eng.tensor_tensor(out=hat[:, kc, c0:c0 + 512], in0=t_s,
                                  in1=ps_bc[:, 512:1024], op=OP.subtract)
        for pool in (sq_pool, xch_pool, a1):
            pool.release()

        # =================== Phase A2: K, V, Q ===================
        # elate: loads needed right after attention (prefetched now)
        elate = tc.alloc_tile_pool(name="elate", bufs=1, side="right")
        wo_s = elate.tile([HD, H, C], BF, tag="wo")
        nc.sync.dma_start(out=wo_s, in_=d_wo[:, :, :])
        xd_s = elate.tile([P, CK, TL + 2], F32, tag="xd")
        nc.sync.dma_start(out=xd_s,
                          in_=d_xd.rearrange("(k p) t -> p k t", p=P))

        kv_state = tc.alloc_tile_pool(name="kvst", bufs=1, side="right")
        st_pool = tc.alloc_tile_pool(name="stage", bufs=3, side="right")
        a2 = tc.alloc_tile_pool(name="a2", bufs=1, side="right")
        wkv_s = a2.tile([P, CK, 2 * C], BF, tag="wkv")
        nc.sync.dma_start(
            out=wkv_s,
            in_=d_wqkv.rearrange("(k p) o -> p k o", p=P)[:, :, C:3 * C],
        )
        kaug = kv_state.tile([HD + 1, H, T], BF, tag="kaug")
        qaug = kv_state.tile([HD + 1, H, TL], BF, tag="qaug")
        vsb = kv_state.tile([P, TK, H, HD + 1], BF, tag="v")

        eng.tensor_copy(out=vsb[:, :, :, HD],
                        in_=ones_b.rearrange("p (g h) -> p g h", h=H)[:, 0:TK, :])
        for h in range(H):
            nc.sync.dma_start(out=kaug[HD:HD + 1, h, :], in_=d_A[:, :])
            nc.sync.dma_start(out=qaug[HD:HD + 1, h, :], in_=d_qA[:, :])

        for n in range(NCH):
            c0 = 512 * n
            for mo in range(CK):
                ps_k = psA.tile([P, 512], F32, tag="mm")
                for kc in range(CK):
                    nc.tensor.matmul(ps_k, wkv_s[:, kc, mo * P:(mo + 1) * P],
                                     hat[:, kc, c0:c0 + 512],
                                     start=(kc == 0), stop=(kc == CK - 1))
                st = st_pool.tile([P, 512], BF, tag="kst")
                eng.tensor_scalar(out=st, in0=ps_k, scalar1=bqkv_s[:, 4 + mo:5 + mo],
                                  scalar2=None, op0=OP.add)
                nc.sync.dma_start(out=kaug[0:HD, 2 * mo, c0:c0 + 512], in_=st[0:HD, :])
                nc.sync.dma_start(out=kaug[0:HD, 2 * mo + 1, c0:c0 + 512], in_=st[HD:P, :])
            for tt in range(4):
                g = 4 * n + tt
                ps_v = psA.tile([P, 512], F32, tag="mm")
                for kc in range(CK):
                    nc.tensor.matmul(ps_v, hat[:, kc, c0 + tt * P:c0 + (tt + 1) * P],
                                     wkv_s[:, kc, C:2 * C],
                                     start=(kc == 0), stop=(kc == CK - 1))
                eng.tensor_tensor(out=vsb[:, g, :, 0:HD],
                                  in0=ps_v.rearrange("p (h d) -> p h d", d=HD),
                                  in1=bvbc_s.rearrange("p (h d) -> p h d", d=HD),
                                  op=OP.add)
        a2.release()
        a2q = tc.alloc_tile_pool(name="a2q", bufs=1, side="right")
        wq_s = a2q.tile([P, CK, C], BF, tag="wq")
        nc.sync.dma_start(
            out=wq_s,
            in_=d_wqkv.rearrange("(k p) o -> p k o", p=P)[:, :, 0:C],
        )
        for mo in range(CK):
            for n2 in range(NL):
                ps_q = psA.tile([P, 512], F32, tag="mm")
                for kc in range(CK):
                    nc.tensor.matmul(ps_q, wq_s[:, kc, mo * P:(mo + 1) * P],
                                     hat[:, kc, bass.ds(j0 + n2 * 512, 512)],
                                     start=(kc == 0), stop=(kc == CK - 1))
                st = st_pool.tile([P, 512], BF, tag="kst")
                eng.tensor_scalar(out=st, in0=ps_q, scalar1=bqkv_s[:, mo:mo + 1],
                                  scalar2=None, op0=OP.add)
                nc.sync.dma_start(out=qaug[0:HD, 2 * mo, n2 * 512:(n2 + 1) * 512],
                                  in_=st[0:HD, :])
                nc.sync.dma_start(out=qaug[0:HD, 2 * mo + 1, n2 * 512:(n2 + 1) * 512],
                                  in_=st[HD:P, :])
        for pool in (a2q, st_pool, hat_pool, psB, psS, psA):
            pool.release()

        # =================== Attention (software-pipelined) ===============
        attn_state = tc.alloc_tile_pool(name="attnst", bufs=1, side="left")
        attnh = attn_state.tile([HD, H, TL], BF, tag="attnh")
        p_pool = tc.alloc_tile_pool(name="pp", bufs=2, side="right")
        psS2 = tc.alloc_tile_pool(name="psS2", bufs=2, space="PSUM")
        psAV = tc.alloc_tile_pool(name="psAV", bufs=2, space="PSUM")

        p_tiles = [None, None]
        av_tiles = [None, None]

        def normalize(h, ps_av):
            for n2 in range(NL):
                cc = slice(n2 * 512, (n2 + 1) * 512)
                den_s = vtmp.tile([1, 512], F32, tag="tmpb")
                eng.tensor_copy(out=den_s, in_=ps_av[HD:HD + 1, cc])
                d_f = vtmp.tile([1, 512], F32, tag="tmpa")
                eng.reciprocal_approx_fast(out=d_f, in_=den_s)
                d_s = vtmp.tile([1, 512], F32R, tag="d")
                eng.tensor_copy(out=d_s, in_=d_f)
                ps_b = psS2.tile([P, 1024], F32, tag="score")
                nc.tensor.matmul(ps_b[0:HD, 0:512], ones_r[0:1, 0:HD],
                                 d_s, start=True, stop=True)
                db_s = tmp_pool.tile([HD, 512], F32, tag="dbs")
                eng.tensor_copy(out=db_s, in_=ps_b[0:HD, 0:512])
                eng.tensor_tensor(out=attnh[:, h, cc], in0=ps_av[0:HD, cc],
                                  in1=db_s, op=OP.mult)

        for h in range(H + 1):
            if h < H:
                p_tiles[h % 2] = p_pool.tile([P, TK, 1024], BF, tag="p", name=f"pbuf{h % 2}")
                av_tiles[h % 2] = psAV.tile([HD + 1, 1024], F32, tag="av", name=f"avbuf{h % 2}")
            for tk in range(TK):
                if h < H:
                    ps_s = psS2.tile([P, 1024], F32, tag="score")
                    for n2 in range(NL):
                        nc.tensor.matmul(ps_s[:, n2 * 512:(n2 + 1) * 512],
                                         kaug[:, h, tk * P:(tk + 1) * P],
                                         qaug[:, h, n2 * 512:(n2 + 1) * 512],
                                         start=True, stop=True)
                    nc.scalar.activation(out=p_tiles[h % 2][:, tk, :], in_=ps_s,
                                         func=AF.Exp)
                if h > 0:
                    pv = p_tiles[(h - 1) % 2]
                    av = av_tiles[(h - 1) % 2]
                    for n2 in range(NL):
                        nc.tensor.matmul(av[:, n2 * 512:(n2 + 1) * 512],
                                         vsb[:, tk, h - 1, :],
                                         pv[:, tk, n2 * 512:(n2 + 1) * 512],
                                         start=(tk == 0), stop=(tk == TK - 1))
            if h > 0:
                normalize(h - 1, av_tiles[(h - 1) % 2])
        for pool in (p_pool, kv_state, psAV, psS2):
            pool.release()

        # =================== late loads ===================
        late = tc.alloc_tile_pool(name="late", bufs=1, side="right")
        xdb_s = late.tile([P, CK, TL + 2], BF, tag="xdb")
        nc.sync.dma_start(out=xdb_s,
                          in_=d_xdb.rearrange("(k p) t -> p k t", p=P))
        ftc2 = late.tile([P, CK, TL], BF, tag="ftc2")
        sq2 = late.tile([P, CK, TL], F16, tag="sq2")
        w1_s = late.tile([P, CK, FF], BF, tag="w1")
        nc.sync.dma_start(out=w1_s,
                          in_=d_w1.rearrange("(k p) o -> p k o", p=P))
        w2_s = late.tile([P, FFK, C], BF, tag="w2")
        nc.sync.dma_start(out=w2_s,
                          in_=d_w2.rearrange("(k p) o -> p k o", p=P))
        pw_s = late.tile([P, CK, C], BF, tag="pw")
        nc.sync.dma_start(out=pw_s,
                          in_=d_pw.rearrange("(k p) o -> p k o", p=P))
        dsa_out = late.tile([P, CK, TL], F32, tag="dsaout")
        z_s = late.tile([P, CK, TL + 2], BF, tag="z")
        z1_s = late.tile([P, CK, TL], BF, tag="z1")
        sqd = late.tile([P, CK, TL + 2], F16, tag="sqd")
        hh_all = late.tile([P, FFK, 512], BF, tag="hhall")

        psC = tc.alloc_tile_pool(name="psC", bufs=2, space="PSUM")
        psH = tc.alloc_tile_pool(name="psH", bufs=2, space="PSUM")
        psS_l = tc.alloc_tile_pool(name="psSl", bufs=1, space="PSUM")
        psB_l = tc.alloc_tile_pool(name="psBl", bufs=1, space="PSUM")

        # ---- out-proj + residual (PE) — runs while DSA DVE chain starts
        for mo in range(CK):
            for n2 in range(NL):
                cc = slice(n2 * 512, (n2 + 1) * 512)
                ps_o = psC.tile([P, 512], F32, tag="mm")
                for h in range(H):
                    nc.tensor.matmul(ps_o, wo_s[:, h, mo * P:(mo + 1) * P],
                                     attnh[:, h, cc],
                                     start=(h == 0), stop=(h == H - 1))
                eng.scalar_tensor_tensor(
                    out=ftc2[:, mo, cc], in0=ps_o, scalar=ob_s[:, mo:mo + 1],
                    in1=xd_s[:, mo, 1 + n2 * 512:1 + (n2 + 1) * 512],
                    op0=OP.add, op1=OP.add)
                eng.tensor_tensor(out=sq2[:, mo, cc], in0=ftc2[:, mo, cc],
                                  in1=ftc2[:, mo, cc], op=OP.mult)
        attn_state.release()

        # ---- DSA squares (DVE)
        for kc in range(CK):
            eng.tensor_tensor(out=sqd[:, kc, :], in0=xdb_s[:, kc, :],
                              in1=xdb_s[:, kc, :], op=OP.mult)

        # ---- DSA LN stats+apply, then LN2 stats+apply (sqrts contiguous)
        for (c0, w) in ((0, 512), (512, 512), (1024, 2)):
            src_b = [xdb_s[:, kc, c0:c0 + w] for kc in range(CK)]
            src_h = [sqd[:, kc, c0:c0 + w] for kc in range(CK)]
            r_s, mr_s = ln_rmr(psS_l, src_b, src_h, w)
            ps_bc = ln_bcast(psB_l, r_s, mr_s, w)
            for kc in range(CK):
                t_s = tmp_pool.tile([P, 512], F32, tag="t")
                eng.tensor_tensor(out=t_s[:, :w], in0=xd_s[:, kc, c0:c0 + w],
                                  in1=ps_bc[:, 0:w], op=OP.mult)
                eng.tensor_tensor(out=t_s[:, :w], in0=t_s[:, :w],
                                  in1=ps_bc[:, 512:512 + w], op=OP.subtract)
                eng.tensor_scalar(out=z_s[:, kc, c0:c0 + w], in0=t_s[:, :w],
                                  scalar1=dsag_s[:, kc:kc + 1], scalar2=dsab_s[:, kc:kc + 1],
                                  op0=OP.mult, op1=OP.add)
        for n2 in range(NL):
            cc = slice(n2 * 512, (n2 + 1) * 512)
            src_b = [ftc2[:, kc, cc] for kc in range(CK)]
            src_h = [sq2[:, kc, cc] for kc in range(CK)]
            r_s, mr_s = ln_rmr(psS_l, src_b, src_h, 512)
            ps_bc = ln_bcast(psB_l, r_s, mr_s, 512)
            for kc in range(CK):
                t_s = tmp_pool.tile([P, 512], F32, tag="t")
                eng.tensor_tensor(out=t_s, in0=ftc2[:, kc, cc],
                                  in1=ps_bc[:, 0:512], op=OP.mult)
                eng.tensor_tensor(out=ftc2[:, kc, cc], in0=t_s,
                                  in1=ps_bc[:, 512:1024], op=OP.subtract)

        # ---- DSA depthwise conv + gelu + pointwise (gelu after all sqrts)
        for i, cpad in enumerate((0, TL + 1)):
            eng.tensor_scalar(out=z_s[:, :, cpad:cpad + 1],
                              in0=z_s[:, :, cpad:cpad + 1],
                              scalar1=mask2_s[:, i:i + 1], scalar2=None, op0=OP.mult)
        for kc in range(CK):
            eng.tensor_scalar(out=z1_s[:, kc, :], in0=z_s[:, kc, 0:TL],
                              scalar1=dw3_s[:, kc, 0:1], scalar2=None, op0=OP.mult)
            eng.scalar_tensor_tensor(out=z1_s[:, kc, :], in0=z_s[:, kc, 1:1 + TL],
                                     scalar=dw3_s[:, kc, 1:2],
                                     in1=z1_s[:, kc, :],
                                     op0=OP.mult, op1=OP.add)
            eng.scalar_tensor_tensor(out=z1_s[:, kc, :], in0=z_s[:, kc, 2:2 + TL],
                                     scalar=dw3_s[:, kc, 2:3],
                                     in1=z1_s[:, kc, :],
                                     op0=OP.mult, op1=OP.add)
            nc.scalar.activation(out=z1_s[:, kc, :], in_=z1_s[:, kc, :],
                                 func=AF.Gelu, bias=dsadb_s[:, kc:kc + 1])
        for mo in range(CK):
            for n2 in range(NL):
                cc = slice(n2 * 512, (n2 + 1) * 512)
                ps_d = psC.tile([P, 512], F32, tag="mm")
                for kc in range(CK):
                    nc.tensor.matmul(ps_d, pw_s[:, kc, mo * P:(mo + 1) * P],
                                     z1_s[:, kc, cc],
                                     start=(kc == 0), stop=(kc == CK - 1))
                eng.tensor_copy(out=dsa_out[:, mo, cc], in_=ps_d)

        # =================== MLP (hh staged in SBUF, 16-chains) ===========
        fin_pool = tc.alloc_tile_pool(name="fin", bufs=3, side="left")
        for n2 in range(NL):
            cc = slice(n2 * 512, (n2 + 1) * 512)
            for ff in range(FFK):
                ps_h = psH.tile([P, 512], F32, tag="mm")
                for kc in range(CK):
                    nc.tensor.matmul(ps_h, w1_s[:, kc, ff * P:(ff + 1) * P],
                                     ftc2[:, kc, cc],
                                     start=(kc == 0), stop=(kc == CK - 1))
                nc.scalar.activation(out=hh_all[:, ff, :], in_=ps_h, func=AF.Gelu,
                                     bias=b1_s[:, ff:ff + 1])
            for mo in range(CK):
                ps_out = psC.tile([P, 512], F32, tag="mm")
                for ff in range(FFK):
                    nc.tensor.matmul(ps_out, w2_s[:, ff, mo * P:(mo + 1) * P],
                                     hh_all[:, ff, :], start=(ff == 0),
                                     stop=(ff == FFK - 1))
                fin_t = fin_pool.tile([P, 512], F32, tag="fin")
                eng.scalar_tensor_tensor(out=fin_t, in0=ps_out,
                                         scalar=bfin_s[:, mo:mo + 1],
                                         in1=dsa_out[:, mo, cc],
                                         op0=OP.add, op1=OP.add)
                nc.sync.dma_start(out=d_out[mo * P:(mo + 1) * P, cc], in_=fin_t)

        for pool in (fin_pool, late, tmp_pool, rmr_pool, vtmp,
                     consts, psB_l, psS_l, psH, psC):
            pool.release()
        elate.release()

    nc.compile()
    return nc


def _in_maps(inputs):
    f = lambda v: np.ascontiguousarray(np.asarray(v), dtype=np.float32)
    bf = lambda v: np.ascontiguousarray(np.asarray(v, dtype=np.float32).astype(ml_dtypes.bfloat16))
    x = f(inputs["x"])            # [B, T, C]
    A = f(inputs["A"])            # [B, T]
    alpha = float(np.asarray(inputs["alpha_bias"]).reshape(-1)[0])
    dst_a = float(np.asarray(inputs["dst_alpha"]))
    dst_b = float(np.asarray(inputs["dst_beta"]))
    conv1_w, conv1_b = f(inputs["conv1_w"]), f(inputs["conv1_b"])
    ln1_g, ln1_b = f(inputs["ln1_g"]), f(inputs["ln1_b"])
    in_w, in_b = f(inputs["in_proj_w"]), f(inputs["in_proj_b"])
    out_w, out_b = f(inputs["out_w"]), f(inputs["out_b"])
    ln2_g, ln2_b = f(inputs["ln2_g"]), f(inputs["ln2_b"])
    w1, b1 = f(inputs["mlp_w1"]), f(inputs["mlp_b1"])
    w2, b2 = f(inputs["mlp_w2"]), f(inputs["mlp_b2"])
    dsa_g, dsa_b = f(inputs["dsa_ln_g"]), f(inputs["dsa_ln_b"])
    dsa_dw, dsa_db = f(inputs["dsa_dw"]), f(inputs["dsa_db"])
    dsa_pw, dsa_pb = f(inputs["dsa_pw"]), f(inputs["dsa_pb"])

    weff = in_w * ln1_g[None, :]
    beff = in_w @ ln1_b + in_b
    weff[:C] /= np.sqrt(HD).astype(np.float32)
    beff[:C] /= np.sqrt(HD).astype(np.float32)
    shared = {
        "convw": bf(np.transpose(conv1_w, (2, 1, 0))),
        "convb": conv1_b,
        "wqkv": bf(weff.T),
        "bqkv": beff,
        "bvbc": np.ascontiguousarray(np.broadcast_to(beff[2 * C:3 * C], (P, C))),
        "wo": bf(out_w.T.reshape(HD * H, C).reshape(H, HD, C).transpose(1, 0, 2)),
        "ob": out_b,
        "w1": bf((w1 * ln2_g[None, :]).T),
        "b1": w1 @ ln2_b + b1,
        "w2": bf((dst_a * w2).T),
        "bfin": dst_a * b2 + dst_b * dsa_pb,
        "pw": bf((dst_b * dsa_pw[:, :, 0]).T),
        "dsag": dsa_g, "dsab": dsa_b,
        "dw3": dsa_dw[:, 0, :], "dsadb": dsa_db,
        "cones": np.ones((P, P), np.float32),
        "conesb": np.ones((P, P), ml_dtypes.bfloat16),
        "cinvC": np.full((P, 1), 1.0 / C, ml_dtypes.bfloat16),
        "cinvCh": np.full((P, 1), 1.0 / C, np.float16),
    }
    maps = []
    for core in range(8):
        b, half = core // 2, core % 2
        j0 = half * TL
        xT = np.zeros((C, T + 2), np.float32)
        xT[:, 1:T + 1] = x[b].T
        xd = np.zeros((C, TL + 2), np.float32)
        lo, hi = j0 - 1, j0 + TL + 1
        slo, shi = max(lo, 0), min(hi, T)
        xd[:, slo - lo:slo - lo + (shi - slo)] = x[b].T[:, slo:shi]
        m = dict(shared)
        m["xT"] = bf(xT)
        m["xd"] = xd
        m["xdb"] = bf(xd)
        mask2 = np.ones((P, 2), np.float32)
        if lo < 0:
            mask2[:, 0] = 0.0
        if hi > T:
            mask2[:, 1] = 0.0
        m["mask2"] = mask2
        m["Arow"] = bf(A[b:b + 1, :])
        m["qArow"] = bf(alpha * A[b:b + 1, j0:j0 + TL])
        m["qoff"] = np.array([[j0]], np.uint32)
        maps.append(m)
    return maps


def _get_program():
    global _CACHED
    if _CACHED is None:
        _CACHED = _build()
    return _CACHED


def kernel(**inputs):
    nc = _get_program()
    maps = _in_maps(inputs)
    res = run_bass_kernel_spmd(nc, maps, list(range(8)))
    out = np.empty((B, T, C), np.float32)
    for core in range(8):
        b, half = core // 2, core % 2
        out[b, half * TL:(half + 1) * TL, :] = res.results[core]["outT"].T
    return out


# revision 35
# speedup vs baseline: 1.4154x; 1.0789x over previous
"""nn_BoundaryGuidedDSTLayer Trainium2 Bass kernel (8-core SPMD, no collectives).

Sharding: core c = (b = c//2, half = c%2). Each core computes the conv
pre-mix + LN1 + K/V over the full T of its batch, and Q / attention /
out-proj / MLP / DSA for its local 1024-column half. Activations are
transposed [C, T]; all big matmuls run in bf16 (fp32 PSUM).

v4: fused A1/A2 pipeline (per-chunk conv -> LN1 -> K/V so the phases
overlap), software-pipelined attention with deeper score-PSUM rotation,
DSA DVE chain overlapped with out-proj/LN2 GEMMs, ff-major MLP.
"""
import sys, os

for _p in ("/opt/trn_rl_repo",):
    if os.path.isdir(_p) and _p not in sys.path:
        sys.path.append(_p)

import numpy as np
import ml_dtypes
import concourse.bass as bass
import concourse.mybir as mybir
import concourse.tile as tile
from concourse.bacc import Bacc
from concourse.bass_utils import run_bass_kernel_spmd

dt = mybir.dt
F32, F32R, U32 = dt.float32, dt.float32r, dt.uint32
BF, F16 = dt.bfloat16, dt.float16
AF = mybir.ActivationFunctionType
OP = mybir.AluOpType

P = 128
B, T, C, H = 4, 2048, 512, 8
HD = C // H          # 64
FF = 4 * C           # 2048
TL = T // 2          # 1024 local columns per core
CK = C // P          # 4
FFK = FF // P        # 16
NCH = T // 512       # 4 chunks over full T
NL = TL // 512       # 2 chunks over local T
TK = T // P          # 16 key tiles

_CACHED = None


def _build():
    nc = Bacc("TRN2", target_bir_lowering=False, debug=False, num_devices=8)

    # ---- DRAM I/O ----
    d_xT = nc.dram_tensor("xT", [C, T + 2], BF, kind="ExternalInput")
    d_xd = nc.dram_tensor("xd", [C, TL + 2], F32, kind="ExternalInput")
    d_xdb = nc.dram_tensor("xdb", [C, TL + 2], BF, kind="ExternalInput")
    d_A = nc.dram_tensor("Arow", [1, T], BF, kind="ExternalInput")
    d_qA = nc.dram_tensor("qArow", [1, TL], BF, kind="ExternalInput")
    d_qoff = nc.dram_tensor("qoff", [1, 1], U32, kind="ExternalInput")
    d_convw = nc.dram_tensor("convw", [3, C, C], BF, kind="ExternalInput")
    d_convb = nc.dram_tensor("convb", [C], F32, kind="ExternalInput")
    d_wqkv = nc.dram_tensor("wqkv", [C, 3 * C], BF, kind="ExternalInput")
    d_bqkv = nc.dram_tensor("bqkv", [3 * C], F32, kind="ExternalInput")
    d_bvbc = nc.dram_tensor("bvbc", [P, C], F32, kind="ExternalInput")
    d_wo = nc.dram_tensor("wo", [HD, H, C], BF, kind="ExternalInput")
    d_ob = nc.dram_tensor("ob", [C], F32, kind="ExternalInput")
    d_w1 = nc.dram_tensor("w1", [C, FF], BF, kind="ExternalInput")
    d_b1 = nc.dram_tensor("b1", [FF], F32, kind="ExternalInput")
    d_w2 = nc.dram_tensor("w2", [FF, C], BF, kind="ExternalInput")
    d_bfin = nc.dram_tensor("bfin", [C], F32, kind="ExternalInput")
    d_pw = nc.dram_tensor("pw", [C, C], BF, kind="ExternalInput")
    d_dsag = nc.dram_tensor("dsag", [C], F32, kind="ExternalInput")
    d_dsab = nc.dram_tensor("dsab", [C], F32, kind="ExternalInput")
    d_dw3 = nc.dram_tensor("dw3", [C, 3], F32, kind="ExternalInput")
    d_dsadb = nc.dram_tensor("dsadb", [C], F32, kind="ExternalInput")
    d_ones = nc.dram_tensor("cones", [P, P], F32, kind="ExternalInput")
    d_onesb = nc.dram_tensor("conesb", [P, P], BF, kind="ExternalInput")
    d_invC = nc.dram_tensor("cinvC", [P, 1], BF, kind="ExternalInput")
    d_invCh = nc.dram_tensor("cinvCh", [P, 1], F16, kind="ExternalInput")
    d_mask2 = nc.dram_tensor("mask2", [P, 2], F32, kind="ExternalInput")
    d_out = nc.dram_tensor("outT", [C, TL], F32, kind="ExternalOutput")

    eng = nc.vector  # DVE for elementwise

    with tile.TileContext(nc) as tc, nc.allow_low_precision(
            reason="bf16 matmuls validated to 4e-3 rel-l2 against fp32 ref"):
        # ---------- persistent small pools ----------
        consts = tc.alloc_tile_pool(name="consts", bufs=1, side="left")
        ones_r = consts.tile([P, P], F32R, tag="ones")
        nc.sync.dma_start(out=ones_r, in_=d_ones[:, :].bitcast(F32R))
        ones_b = consts.tile([P, P], BF, tag="onesb")
        nc.sync.dma_start(out=ones_b, in_=d_onesb[:, :])
        invC_b = consts.tile([P, 1], BF, tag="invC")
        nc.sync.dma_start(out=invC_b, in_=d_invC[:, :])
        invC_h = consts.tile([P, 1], F16, tag="invCh")
        nc.sync.dma_start(out=invC_h, in_=d_invCh[:, :])
        convb_s = consts.tile([P, CK], F32, tag="convb")
        nc.sync.dma_start(out=convb_s, in_=d_convb.rearrange("(m p) -> p m", p=P))
        bqkv_s = consts.tile([P, 12], F32, tag="bqkv")
        nc.sync.dma_start(out=bqkv_s, in_=d_bqkv.rearrange("(m p) -> p m", p=P))
        ob_s = consts.tile([P, CK], F32, tag="ob")
        nc.sync.dma_start(out=ob_s, in_=d_ob.rearrange("(m p) -> p m", p=P))
        b1_s = consts.tile([P, FFK], F32, tag="b1")
        nc.sync.dma_start(out=b1_s, in_=d_b1.rearrange("(m p) -> p m", p=P))
        bfin_s = consts.tile([P, CK], F32, tag="bfin")
        nc.sync.dma_start(out=bfin_s, in_=d_bfin.rearrange("(m p) -> p m", p=P))
        dsag_s = consts.tile([P, CK], F32, tag="dsag")
        nc.sync.dma_start(out=dsag_s, in_=d_dsag.rearrange("(m p) -> p m", p=P))
        dsab_s = consts.tile([P, CK], F32, tag="dsab")
        nc.sync.dma_start(out=dsab_s, in_=d_dsab.rearrange("(m p) -> p m", p=P))
        dw3_s = consts.tile([P, CK, 3], F32, tag="dw3")
        nc.sync.dma_start(out=dw3_s, in_=d_dw3.rearrange("(m p) d -> p m d", p=P))
        dsadb_s = consts.tile([P, CK], F32, tag="dsadb")
        nc.sync.dma_start(out=dsadb_s, in_=d_dsadb.rearrange("(m p) -> p m", p=P))
        bvbc_s = consts.tile([P, C], F32, tag="bvbc")
        nc.sync.dma_start(out=bvbc_s, in_=d_bvbc[:, :])
        mask2_s = consts.tile([P, 2], F32, tag="mask2")
        nc.sync.dma_start(out=mask2_s, in_=d_mask2[:, :])
        qoff_s = consts.tile([1, 1], U32, tag="qoff")
        nc.sync.dma_start(out=qoff_s, in_=d_qoff[:, :])
        regs = nc.alloc_registers("qoffr")
        nc.regs_load(regs, qoff_s[0:1, 0:1])
        j0 = nc.snap(regs, donate=True, min_val=0, max_val=TL)

        EPS = 1e-5

        vtmp = tc.alloc_tile_pool(name="vtmp", bufs=2, side="left")
        rmr_pool = tc.alloc_tile_pool(name="rmr", bufs=2, side="left")
        tmp_pool = tc.alloc_tile_pool(name="tmp", bufs=2, side="left")
        hat_pool = tc.alloc_tile_pool(name="hatp", bufs=1, side="left")
        hat = hat_pool.tile([P, CK, T], BF, tag="hat")

        def ln_rmr(psS, src_b, src_h, n_cols):
            ps_mean = psS.tile([1, 512], F32, tag="mean")
            for kc in range(CK):
                nc.tensor.matmul(ps_mean[0:1, :n_cols], invC_b[:, :], src_b[kc],
                                 start=(kc == 0), stop=(kc == CK - 1))
            ps_ex2 = psS.tile([1, 512], F32, tag="ex2")
            for kc in range(CK):
                nc.tensor.matmul(ps_ex2[0:1, :n_cols], invC_h[:, :], src_h[kc],
                                 start=(kc == 0), stop=(kc == CK - 1))
            m_s = vtmp.tile([1, 512], F32, tag="m")
            eng.tensor_copy(out=m_s[:, :n_cols], in_=ps_mean[0:1, :n_cols])
            m2_s = vtmp.tile([1, 512], F32, tag="tmpa")
            eng.tensor_tensor(out=m2_s[:, :n_cols], in0=m_s[:, :n_cols],
                              in1=m_s[:, :n_cols], op=OP.mult)
            vpe_s = vtmp.tile([1, 512], F32, tag="tmpa")
            eng.scalar_tensor_tensor(out=vpe_s[:, :n_cols], in0=ps_ex2[0:1, :n_cols],
                                     scalar=EPS, in1=m2_s[:, :n_cols],
                                     op0=OP.add, op1=OP.subtract)
            std_s = vtmp.tile([1, 512], F32, tag="tmpb")
            nc.scalar.activation(out=std_s[:, :n_cols], in_=vpe_s[:, :n_cols],
                                 func=AF.Sqrt)
            r_f = vtmp.tile([1, 512], F32, tag="tmpb")
            eng.reciprocal_approx_fast(out=r_f[:, :n_cols], in_=std_s[:, :n_cols])
            r_s = rmr_pool.tile([1, 512], F32R, tag="r")
            eng.tensor_copy(out=r_s[:, :n_cols], in_=r_f[:, :n_cols])
            mr_s = rmr_pool.tile([1, 512], F32R, tag="mr")
            eng.tensor_tensor(out=mr_s[:, :n_cols], in0=m_s[:, :n_cols],
                              in1=r_f[:, :n_cols], op=OP.mult)
            return r_s, mr_s

        def ln_bcast(psB, r_s, mr_s, n_cols):
            ps_bc = psB.tile([P, 1024], F32, tag="lnbc")
            nc.tensor.matmul(ps_bc[:, 0:n_cols], ones_r[0:1, :],
                             r_s[:, :n_cols], start=True, stop=True)
            nc.tensor.matmul(ps_bc[:, 512:512 + n_cols], ones_r[0:1, :],
                             mr_s[:, :n_cols], start=True, stop=True)
            return ps_bc

        # ============ Fused A1/A2: per chunk conv -> LN1 -> K/V ============
        # right-side pools allocated up front so K/V weights prefetch early
        elate = tc.alloc_tile_pool(name="elate", bufs=1, side="right")
        wo_s = elate.tile([HD, H, C], BF, tag="wo")
        nc.sync.dma_start(out=wo_s, in_=d_wo[:, :, :])
        xd_s = elate.tile([P, CK, TL + 2], F32, tag="xd")
        nc.sync.dma_start(out=xd_s,
                          in_=d_xd.rearrange("(k p) t -> p k t", p=P))
        kv_state = tc.alloc_tile_pool(name="kvst", bufs=1, side="right")
        kaug = kv_state.tile([HD + 1, H, T], BF, tag="kaug")
        qaug = kv_state.tile([HD + 1, H, TL], BF, tag="qaug")
        vsb = kv_state.tile([P, TK, H, HD + 1], BF, tag="v")
        st_pool = tc.alloc_tile_pool(name="stage", bufs=3, side="right")
        a2 = tc.alloc_tile_pool(name="a2", bufs=1, side="right")
        wkv_s = a2.tile([P, CK, 2 * C], BF, tag="wkv")
        nc.sync.dma_start(
            out=wkv_s,
            in_=d_wqkv.rearrange("(k p) o -> p k o", p=P)[:, :, C:3 * C],
        )
        wq_s = a2.tile([P, CK, C], BF, tag="wq")
        nc.sync.dma_start(
            out=wq_s,
            in_=d_wqkv.rearrange("(k p) o -> p k o", p=P)[:, :, 0:C],
        )
        eng.tensor_copy(out=vsb[:, :, :, HD],
                        in_=ones_b.rearrange("p (g h) -> p g h", h=H)[:, 0:TK, :])
        for h in range(H):
            nc.sync.dma_start(out=kaug[HD:HD + 1, h, :], in_=d_A[:, :])
            nc.sync.dma_start(out=qaug[HD:HD + 1, h, :], in_=d_qA[:, :])

        a1 = tc.alloc_tile_pool(name="a1", bufs=1, side="left")
        convw_s = a1.tile([P, 3, CK, C], BF, tag="convw")
        nc.sync.dma_start(
            out=convw_s,
            in_=d_convw.rearrange("d (k p) o -> p d k o", p=P),
        )
        xch_pool = tc.alloc_tile_pool(name="xch", bufs=3, side="left")
        sq_pool = tc.alloc_tile_pool(name="sq", bufs=1, side="left")
        psA = tc.alloc_tile_pool(name="psA", bufs=4, space="PSUM")
        psS = tc.alloc_tile_pool(name="psS", bufs=1, space="PSUM")
        psB = tc.alloc_tile_pool(name="psB", bufs=1, space="PSUM")

        sqs = sq_pool.tile([P, CK, 512], F16, tag="sqs")
        for n in range(NCH):
            c0 = 512 * n
            x_ch = xch_pool.tile([P, CK, 514], BF, tag="xch")
            nc.sync.dma_start(
                out=x_ch,
                in_=d_xT[:, c0:c0 + 514].rearrange("(k p) t -> p k t", p=P),
            )
            # conv + gelu + residual + squares for this chunk
            for mo in range(CK):
                ps_c = psA.tile([P, 512], F32, tag="mm")
                first = True
                for dtap in range(3):
                    for kc in range(CK):
                        nc.tensor.matmul(
                            ps_c,
                            convw_s[:, dtap, kc, mo * P:(mo + 1) * P],
                            x_ch[:, kc, dtap:dtap + 512],
                            start=first, stop=(dtap == 2 and kc == CK - 1),
                        )
                        first = False
                f_t = hat[:, mo, c0:c0 + 512]
                nc.scalar.activation(out=f_t, in_=ps_c, func=AF.Gelu,
                                     bias=convb_s[:, mo:mo + 1])
                eng.tensor_tensor(out=f_t, in0=f_t,
                                  in1=x_ch[:, mo, 1:513], op=OP.add)
                eng.tensor_tensor(out=sqs[:, mo, :], in0=f_t,
                                  in1=f_t, op=OP.mult)
            # LN1 stats + apply for this chunk
            src_b = [hat[:, kc, c0:c0 + 512] for kc in range(CK)]
            src_h = [sqs[:, kc, :] for kc in range(CK)]
            r_s, mr_s = ln_rmr(psS, src_b, src_h, 512)
            ps_bc = ln_bcast(psB, r_s, mr_s, 512)
            for kc in range(CK):
                t_s = tmp_pool.tile([P, 512], F32, tag="t")
                eng.tensor_tensor(out=t_s, in0=hat[:, kc, c0:c0 + 512],
                                  in1=ps_bc[:, 0:512], op=OP.mult)
                eng.tensor_tensor(out=hat[:, kc, c0:c0 + 512], in0=t_s,
                                  in1=ps_bc[:, 512:1024], op=OP.subtract)
            # K for this chunk
            for mo in range(CK):
                ps_k = psA.tile([P, 512], F32, tag="mm")
                for kc in range(CK):
                    nc.tensor.matmul(ps_k, wkv_s[:, kc, mo * P:(mo + 1) * P],
                                     hat[:, kc, c0:c0 + 512],
                                     start=(kc == 0), stop=(kc == CK - 1))
                st = st_pool.tile([P, 512], BF, tag="kst")
                eng.tensor_scalar(out=st, in0=ps_k, scalar1=bqkv_s[:, 4 + mo:5 + mo],
                                  scalar2=None, op0=OP.add)
                nc.sync.dma_start(out=kaug[0:HD, 2 * mo, c0:c0 + 512], in_=st[0:HD, :])
                nc.sync.dma_start(out=kaug[0:HD, 2 * mo + 1, c0:c0 + 512], in_=st[HD:P, :])
            # V for this chunk
            for tt in range(4):
                g = 4 * n + tt
                ps_v = psA.tile([P, 512], F32, tag="mm")
                for kc in range(CK):
                    nc.tensor.matmul(ps_v, hat[:, kc, c0 + tt * P:c0 + (tt + 1) * P],
                                     wkv_s[:, kc, C:2 * C],
                                     start=(kc == 0), stop=(kc == CK - 1))
                eng.tensor_tensor(out=vsb[:, g, :, 0:HD],
                                  in0=ps_v.rearrange("p (h d) -> p h d", d=HD),
                                  in1=bvbc_s.rearrange("p (h d) -> p h d", d=HD),
                                  op=OP.add)
        # Q (local half via dynamic offset)
        for mo in range(CK):
            for n2 in range(NL):
                ps_q = psA.tile([P, 512], F32, tag="mm")
                for kc in range(CK):
                    nc.tensor.matmul(ps_q, wq_s[:, kc, mo * P:(mo + 1) * P],
                                     hat[:, kc, bass.ds(j0 + n2 * 512, 512)],
                                     start=(kc == 0), stop=(kc == CK - 1))
                st = st_pool.tile([P, 512], BF, tag="kst")
                eng.tensor_scalar(out=st, in0=ps_q, scalar1=bqkv_s[:, mo:mo + 1],
                                  scalar2=None, op0=OP.add)
                nc.sync.dma_start(out=qaug[0:HD, 2 * mo, n2 * 512:(n2 + 1) * 512],
                                  in_=st[0:HD, :])
                nc.sync.dma_start(out=qaug[0:HD, 2 * mo + 1, n2 * 512:(n2 + 1) * 512],
                                  in_=st[HD:P, :])
        for pool in (sq_pool, xch_pool, a1):
            pool.release()
        for pool in (a2, st_pool, hat_pool, psB, psS, psA):
            pool.release()

        # =================== Attention (software-pipelined) ===============
        attn_state = tc.alloc_tile_pool(name="attnst", bufs=1, side="left")
        attnh = attn_state.tile([HD, H, TL], BF, tag="attnh")
        p_pool = tc.alloc_tile_pool(name="pp", bufs=2, side="right")
        psS2 = tc.alloc_tile_pool(name="psS2", bufs=3, space="PSUM")
        psAV = tc.alloc_tile_pool(name="psAV", bufs=1, space="PSUM")

        p_tiles = [None, None]
        av_tiles = [None, None]

        def normalize(h, ps_av):
            for n2 in range(NL):
                cc = slice(n2 * 512, (n2 + 1) * 512)
                den_s = vtmp.tile([1, 512], F32, tag="tmpb")
                eng.tensor_copy(out=den_s, in_=ps_av[HD:HD + 1, cc])
                d_f = vtmp.tile([1, 512], F32, tag="tmpa")
                eng.reciprocal_approx_fast(out=d_f, in_=den_s)
                d_s = vtmp.tile([1, 512], F32R, tag="d")
                eng.tensor_copy(out=d_s, in_=d_f)
                ps_b = psS2.tile([P, 1024], F32, tag="score")
                nc.tensor.matmul(ps_b[0:HD, 0:512], ones_r[0:1, 0:HD],
                                 d_s, start=True, stop=True)
                db_s = tmp_pool.tile([HD, 512], F32, tag="dbs")
                eng.tensor_copy(out=db_s, in_=ps_b[0:HD, 0:512])
                eng.tensor_tensor(out=attnh[:, h, cc], in0=ps_av[0:HD, cc],
                                  in1=db_s, op=OP.mult)

        for h in range(H + 1):
            if h < H:
                p_tiles[h % 2] = p_pool.tile([P, TK, 1024], BF, tag="p",
                                             name=f"pbuf{h % 2}")
                av_tiles[h % 2] = psAV.tile([HD + 1, 1024], F32, tag="av",
                                            name=f"avbuf{h % 2}")
            for tk in range(TK):
                if h < H:
                    ps_s = psS2.tile([P, 1024], F32, tag="score")
                    for n2 in range(NL):
                        nc.tensor.matmul(ps_s[:, n2 * 512:(n2 + 1) * 512],
                                         kaug[:, h, tk * P:(tk + 1) * P],
                                         qaug[:, h, n2 * 512:(n2 + 1) * 512],
                                         start=True, stop=True)
                    nc.scalar.activation(out=p_tiles[h % 2][:, tk, :], in_=ps_s,
                                         func=AF.Exp)
                if h > 0:
                    pv = p_tiles[(h - 1) % 2]
                    av = av_tiles[(h - 1) % 2]
                    for n2 in range(NL):
                        nc.tensor.matmul(av[:, n2 * 512:(n2 + 1) * 512],
                                         vsb[:, tk, h - 1, :],
                                         pv[:, tk, n2 * 512:(n2 + 1) * 512],
                                         start=(tk == 0), stop=(tk == TK - 1))
            if h > 0:
                normalize(h - 1, av_tiles[(h - 1) % 2])
        for pool in (p_pool, kv_state, psAV, psS2):
            pool.release()

        # =================== late loads ===================
        late = tc.alloc_tile_pool(name="late", bufs=1, side="right")
        xdb_s = late.tile([P, CK, TL + 2], BF, tag="xdb")
        nc.sync.dma_start(out=xdb_s,
                          in_=d_xdb.rearrange("(k p) t -> p k t", p=P))
        ftc2 = late.tile([P, CK, TL], BF, tag="ftc2")
        sq2 = late.tile([P, CK, TL], F16, tag="sq2")
        w1_s = late.tile([P, CK, FF], BF, tag="w1")
        nc.sync.dma_start(out=w1_s,
                          in_=d_w1.rearrange("(k p) o -> p k o", p=P))
        w2_s = late.tile([P, FFK, C], BF, tag="w2")
        nc.sync.dma_start(out=w2_s,
                          in_=d_w2.rearrange("(k p) o -> p k o", p=P))
        pw_s = late.tile([P, CK, C], BF, tag="pw")
        nc.sync.dma_start(out=pw_s,
                          in_=d_pw.rearrange("(k p) o -> p k o", p=P))
        dsa_out = late.tile([P, CK, TL], F32, tag="dsaout")
        z_s = late.tile([P, CK, TL + 2], BF, tag="z")
        z1_s = late.tile([P, CK, TL], BF, tag="z1")
        sqd = late.tile([P, CK, TL + 2], F16, tag="sqd")

        psC = tc.alloc_tile_pool(name="psC", bufs=3, space="PSUM")
        psS_l = tc.alloc_tile_pool(name="psSl", bufs=1, space="PSUM")
        psB_l = tc.alloc_tile_pool(name="psBl", bufs=1, space="PSUM")

        # ---- out-proj + residual (PE) — runs while DSA DVE chain starts
        for mo in range(CK):
            for n2 in range(NL):
                cc = slice(n2 * 512, (n2 + 1) * 512)
                ps_o = psC.tile([P, 512], F32, tag="mm")
                for h in range(H):
                    nc.tensor.matmul(ps_o, wo_s[:, h, mo * P:(mo + 1) * P],
                                     attnh[:, h, cc],
                                     start=(h == 0), stop=(h == H - 1))
                eng.scalar_tensor_tensor(
                    out=ftc2[:, mo, cc], in0=ps_o, scalar=ob_s[:, mo:mo + 1],
                    in1=xd_s[:, mo, 1 + n2 * 512:1 + (n2 + 1) * 512],
                    op0=OP.add, op1=OP.add)
                eng.tensor_tensor(out=sq2[:, mo, cc], in0=ftc2[:, mo, cc],
                                  in1=ftc2[:, mo, cc], op=OP.mult)
        attn_state.release()

        # ---- DSA squares (DVE)
        for kc in range(CK):
            eng.tensor_tensor(out=sqd[:, kc, :], in0=xdb_s[:, kc, :],
                              in1=xdb_s[:, kc, :], op=OP.mult)

        # ---- DSA LN stats+apply, then LN2 stats+apply (sqrts contiguous)
        for (c0, w) in ((0, 512), (512, 512), (1024, 2)):
            src_b = [xdb_s[:, kc, c0:c0 + w] for kc in range(CK)]
            src_h = [sqd[:, kc, c0:c0 + w] for kc in range(CK)]
            r_s, mr_s = ln_rmr(psS_l, src_b, src_h, w)
            ps_bc = ln_bcast(psB_l, r_s, mr_s, w)
            for kc in range(CK):
                t_s = tmp_pool.tile([P, 512], F32, tag="t")
                eng.tensor_tensor(out=t_s[:, :w], in0=xd_s[:, kc, c0:c0 + w],
                                  in1=ps_bc[:, 0:w], op=OP.mult)
                eng.tensor_tensor(out=t_s[:, :w], in0=t_s[:, :w],
                                  in1=ps_bc[:, 512:512 + w], op=OP.subtract)
                eng.tensor_scalar(out=z_s[:, kc, c0:c0 + w], in0=t_s[:, :w],
                                  scalar1=dsag_s[:, kc:kc + 1], scalar2=dsab_s[:, kc:kc + 1],
                                  op0=OP.mult, op1=OP.add)
        for n2 in range(NL):
            cc = slice(n2 * 512, (n2 + 1) * 512)
            src_b = [ftc2[:, kc, cc] for kc in range(CK)]
            src_h = [sq2[:, kc, cc] for kc in range(CK)]
            r_s, mr_s = ln_rmr(psS_l, src_b, src_h, 512)
            ps_bc = ln_bcast(psB_l, r_s, mr_s, 512)
            for kc in range(CK):
                t_s = tmp_pool.tile([P, 512], F32, tag="t")
                eng.tensor_tensor(out=t_s, in0=ftc2[:, kc, cc],
                                  in1=ps_bc[:, 0:512], op=OP.mult)
                eng.tensor_tensor(out=ftc2[:, kc, cc], in0=t_s,
                                  in1=ps_bc[:, 512:1024], op=OP.subtract)
        for pool in (psB_l, psS_l):
            pool.release()
        psO = tc.alloc_tile_pool(name="psO", bufs=1, space="PSUM")

        # ---- DSA depthwise conv + gelu + pointwise
        for i, cpad in enumerate((0, TL + 1)):
            eng.tensor_scalar(out=z_s[:, :, cpad:cpad + 1],
                              in0=z_s[:, :, cpad:cpad + 1],
                              scalar1=mask2_s[:, i:i + 1], scalar2=None, op0=OP.mult)
        for kc in range(CK):
            eng.tensor_scalar(out=z1_s[:, kc, :], in0=z_s[:, kc, 0:TL],
                              scalar1=dw3_s[:, kc, 0:1], scalar2=None, op0=OP.mult)
            eng.scalar_tensor_tensor(out=z1_s[:, kc, :], in0=z_s[:, kc, 1:1 + TL],
                                     scalar=dw3_s[:, kc, 1:2],
                                     in1=z1_s[:, kc, :],
                                     op0=OP.mult, op1=OP.add)
            eng.scalar_tensor_tensor(out=z1_s[:, kc, :], in0=z_s[:, kc, 2:2 + TL],
                                     scalar=dw3_s[:, kc, 2:3],
                                     in1=z1_s[:, kc, :],
                                     op0=OP.mult, op1=OP.add)
            nc.scalar.activation(out=z1_s[:, kc, :], in_=z1_s[:, kc, :],
                                 func=AF.Gelu, bias=dsadb_s[:, kc:kc + 1])
        for mo in range(CK):
            for n2 in range(NL):
                cc = slice(n2 * 512, (n2 + 1) * 512)
                ps_d = psC.tile([P, 512], F32, tag="mm")
                for kc in range(CK):
                    nc.tensor.matmul(ps_d, pw_s[:, kc, mo * P:(mo + 1) * P],
                                     z1_s[:, kc, cc],
                                     start=(kc == 0), stop=(kc == CK - 1))
                eng.tensor_copy(out=dsa_out[:, mo, cc], in_=ps_d)

        # =================== MLP (ff-major, 4 accumulators) ===============
        hh_pool = tc.alloc_tile_pool(name="hh", bufs=3, side="left")
        fin_pool = tc.alloc_tile_pool(name="fin", bufs=3, side="left")
        for n2 in range(NL):
            cc = slice(n2 * 512, (n2 + 1) * 512)
            ps_out = [psO.tile([P, 512], F32, tag=f"out{mo}", name=f"psout{mo}")
                      for mo in range(CK)]
            for ff in range(FFK):
                ps_h = psC.tile([P, 512], F32, tag="mm")
                for kc in range(CK):
                    nc.tensor.matmul(ps_h, w1_s[:, kc, ff * P:(ff + 1) * P],
                                     ftc2[:, kc, cc],
                                     start=(kc == 0), stop=(kc == CK - 1))
                hh_t = hh_pool.tile([P, 512], BF, tag="hh")
                nc.scalar.activation(out=hh_t, in_=ps_h, func=AF.Gelu,
                                     bias=b1_s[:, ff:ff + 1])
                for mo in range(CK):
                    nc.tensor.matmul(ps_out[mo], w2_s[:, ff, mo * P:(mo + 1) * P],
                                     hh_t, start=(ff == 0), stop=(ff == FFK - 1))
            for mo in range(CK):
                fin_t = fin_pool.tile([P, 512], F32, tag="fin")
                eng.scalar_tensor_tensor(out=fin_t, in0=ps_out[mo],
                                         scalar=bfin_s[:, mo:mo + 1],
                                         in1=dsa_out[:, mo, cc],
                                         op0=OP.add, op1=OP.add)
                nc.sync.dma_start(out=d_out[mo * P:(mo + 1) * P, cc], in_=fin_t)

        for pool in (fin_pool, hh_pool, late, tmp_pool, rmr_pool, vtmp,
                     consts, psO, psC):
            pool.release()
        elate.release()

    nc.compile()
    return nc


def _in_maps(inputs):
    f = lambda v: np.ascontiguousarray(np.asarray(v), dtype=np.float32)
    bf = lambda v: np.ascontiguousarray(np.asarray(v, dtype=np.float32).astype(ml_dtypes.bfloat16))
    x = f(inputs["x"])            # [B, T, C]
    A = f(inputs["A"])            # [B, T]
    alpha = float(np.asarray(inputs["alpha_bias"]).reshape(-1)[0])
    dst_a = float(np.asarray(inputs["dst_alpha"]))
    dst_b = float(np.asarray(inputs["dst_beta"]))
    conv1_w, conv1_b = f(inputs["conv1_w"]), f(inputs["conv1_b"])
    ln1_g, ln1_b = f(inputs["ln1_g"]), f(inputs["ln1_b"])
    in_w, in_b = f(inputs["in_proj_w"]), f(inputs["in_proj_b"])
    out_w, out_b = f(inputs["out_w"]), f(inputs["out_b"])
    ln2_g, ln2_b = f(inputs["ln2_g"]), f(inputs["ln2_b"])
    w1, b1 = f(inputs["mlp_w1"]), f(inputs["mlp_b1"])
    w2, b2 = f(inputs["mlp_w2"]), f(inputs["mlp_b2"])
    dsa_g, dsa_b = f(inputs["dsa_ln_g"]), f(inputs["dsa_ln_b"])
    dsa_dw, dsa_db = f(inputs["dsa_dw"]), f(inputs["dsa_db"])
    dsa_pw, dsa_pb = f(inputs["dsa_pw"]), f(inputs["dsa_pb"])

    weff = in_w * ln1_g[None, :]
    beff = in_w @ ln1_b + in_b
    weff[:C] /= np.sqrt(HD).astype(np.float32)
    beff[:C] /= np.sqrt(HD).astype(np.float32)
    shared = {
        "convw": bf(np.transpose(conv1_w, (2, 1, 0))),
        "convb": conv1_b,
        "wqkv": bf(weff.T),
        "bqkv": beff,
        "bvbc": np.ascontiguousarray(np.broadcast_to(beff[2 * C:3 * C], (P, C))),
        "wo": bf(out_w.T.reshape(HD * H, C).reshape(H, HD, C).transpose(1, 0, 2)),
        "ob": out_b,
        "w1": bf((w1 * ln2_g[None, :]).T),
        "b1": w1 @ ln2_b + b1,
        "w2": bf((dst_a * w2).T),
        "bfin": dst_a * b2 + dst_b * dsa_pb,
        "pw": bf((dst_b * dsa_pw[:, :, 0]).T),
        "dsag": dsa_g, "dsab": dsa_b,
        "dw3": dsa_dw[:, 0, :], "dsadb": dsa_db,
        "cones": np.ones((P, P), np.float32),
        "conesb": np.ones((P, P), ml_dtypes.bfloat16),
        "cinvC": np.full((P, 1), 1.0 / C, ml_dtypes.bfloat16),
        "cinvCh": np.full((P, 1), 1.0 / C, np.float16),
    }
    maps = []
    for core in range(8):
        b, half = core // 2, core % 2
        j0 = half * TL
        xT = np.zeros((C, T + 2), np.float32)
        xT[:, 1:T + 1] = x[b].T
        xd = np.zeros((C, TL + 2), np.float32)
        lo, hi = j0 - 1, j0 + TL + 1
        slo, shi = max(lo, 0), min(hi, T)
        xd[:, slo - lo:slo - lo + (shi - slo)] = x[b].T[:, slo:shi]
        m = dict(shared)
        m["xT"] = bf(xT)
        m["xd"] = xd
        m["xdb"] = bf(xd)
        mask2 = np.ones((P, 2), np.float32)
        if lo < 0:
            mask2[:, 0] = 0.0
        if hi > T:
            mask2[:, 1] = 0.0
        m["mask2"] = mask2
        m["Arow"] = bf(A[b:b + 1, :])
        m["qArow"] = bf(alpha * A[b:b + 1, j0:j0 + TL])
        m["qoff"] = np.array([[j0]], np.uint32)
        maps.append(m)
    return maps


def _get_program():
    global _CACHED
    if _CACHED is None:
        _CACHED = _build()
    return _CACHED


def kernel(**inputs):
    nc = _get_program()
    maps = _in_maps(inputs)
    res = run_bass_kernel_spmd(nc, maps, list(range(8)))
    out = np.empty((B, T, C), np.float32)
    for core in range(8):
        b, half = core // 2, core % 2
        out[b, half * TL:(half + 1) * TL, :] = res.results[core]["outT"].T
    return out


# revision 37
# speedup vs baseline: 1.5590x; 1.1014x over previous
"""nn_BoundaryGuidedDSTLayer Trainium2 Bass kernel (8-core SPMD, no collectives).

Sharding: core c = (b = c//2, half = c%2). Each core computes the conv
pre-mix + LN1 + K/V over the full T of its batch, and Q / attention /
out-proj / MLP / DSA for its local 1024-column half. Activations are
transposed [C, T]; all big matmuls run in bf16 (fp32 PSUM).

v4: fused A1/A2 pipeline (per-chunk conv -> LN1 -> K/V so the phases
overlap), software-pipelined attention with deeper score-PSUM rotation,
DSA DVE chain overlapped with out-proj/LN2 GEMMs, ff-major MLP.
"""
import sys, os

for _p in ("/opt/trn_rl_repo",):
    if os.path.isdir(_p) and _p not in sys.path:
        sys.path.append(_p)

import numpy as np
import ml_dtypes
import concourse.bass as bass
import concourse.mybir as mybir
import concourse.tile as tile
from concourse.bacc import Bacc
from concourse.bass_utils import run_bass_kernel_spmd

dt = mybir.dt
F32, F32R, U32 = dt.float32, dt.float32r, dt.uint32
BF, F16 = dt.bfloat16, dt.float16
AF = mybir.ActivationFunctionType
OP = mybir.AluOpType

P = 128
B, T, C, H = 4, 2048, 512, 8
HD = C // H          # 64
FF = 4 * C           # 2048
TL = T // 2          # 1024 local columns per core
CK = C // P          # 4
FFK = FF // P        # 16
NCH = T // 512       # 4 chunks over full T
NL = TL // 512       # 2 chunks over local T
TK = T // P          # 16 key tiles

_CACHED = None


def _build():
    nc = Bacc("TRN2", target_bir_lowering=False, debug=False, num_devices=8)

    # ---- DRAM I/O ----
    d_xT = nc.dram_tensor("xT", [C, T + 2], BF, kind="ExternalInput")
    d_xd = nc.dram_tensor("xd", [C, TL + 2], F32, kind="ExternalInput")
    d_xdb = nc.dram_tensor("xdb", [C, TL + 2], BF, kind="ExternalInput")
    d_A = nc.dram_tensor("Arow", [1, T], BF, kind="ExternalInput")
    d_qA = nc.dram_tensor("qArow", [1, TL], BF, kind="ExternalInput")
    d_qoff = nc.dram_tensor("qoff", [1, 1], U32, kind="ExternalInput")
    d_convw = nc.dram_tensor("convw", [3, C, C], BF, kind="ExternalInput")
    d_convb = nc.dram_tensor("convb", [C], F32, kind="ExternalInput")
    d_wqkv = nc.dram_tensor("wqkv", [C, 3 * C], BF, kind="ExternalInput")
    d_bqkv = nc.dram_tensor("bqkv", [3 * C], F32, kind="ExternalInput")
    d_bvbc = nc.dram_tensor("bvbc", [P, C], F32, kind="ExternalInput")
    d_wo = nc.dram_tensor("wo", [HD, H, C], BF, kind="ExternalInput")
    d_ob = nc.dram_tensor("ob", [C], F32, kind="ExternalInput")
    d_w1 = nc.dram_tensor("w1", [C, FF], BF, kind="ExternalInput")
    d_b1 = nc.dram_tensor("b1", [FF], F32, kind="ExternalInput")
    d_w2 = nc.dram_tensor("w2", [FF, C], BF, kind="ExternalInput")
    d_bfin = nc.dram_tensor("bfin", [C], F32, kind="ExternalInput")
    d_pw = nc.dram_tensor("pw", [C, C], BF, kind="ExternalInput")
    d_dsag = nc.dram_tensor("dsag", [C], F32, kind="ExternalInput")
    d_dsab = nc.dram_tensor("dsab", [C], F32, kind="ExternalInput")
    d_dw3 = nc.dram_tensor("dw3", [C, 3], F32, kind="ExternalInput")
    d_dsadb = nc.dram_tensor("dsadb", [C], F32, kind="ExternalInput")
    d_ones = nc.dram_tensor("cones", [P, P], F32, kind="ExternalInput")
    d_onesb = nc.dram_tensor("conesb", [P, P], BF, kind="ExternalInput")
    d_invC = nc.dram_tensor("cinvC", [P, 1], BF, kind="ExternalInput")
    d_invCh = nc.dram_tensor("cinvCh", [P, 1], F16, kind="ExternalInput")
    d_mask2 = nc.dram_tensor("mask2", [P, 2], F32, kind="ExternalInput")
    d_out = nc.dram_tensor("outT", [C, TL], F32, kind="ExternalOutput")

    eng = nc.vector  # DVE for elementwise

    with tile.TileContext(nc) as tc, nc.allow_low_precision(
            reason="bf16 matmuls validated to 4e-3 rel-l2 against fp32 ref"):
        # ---------- persistent small pools ----------
        consts = tc.alloc_tile_pool(name="consts", bufs=1, side="left")
        ones_r = consts.tile([P, P], F32R, tag="ones")
        nc.sync.dma_start(out=ones_r, in_=d_ones[:, :].bitcast(F32R))
        ones_b = consts.tile([P, P], BF, tag="onesb")
        nc.sync.dma_start(out=ones_b, in_=d_onesb[:, :])
        invC_b = consts.tile([P, 1], BF, tag="invC")
        nc.sync.dma_start(out=invC_b, in_=d_invC[:, :])
        invC_h = consts.tile([P, 1], F16, tag="invCh")
        nc.sync.dma_start(out=invC_h, in_=d_invCh[:, :])
        convb_s = consts.tile([P, CK], F32, tag="convb")
        nc.sync.dma_start(out=convb_s, in_=d_convb.rearrange("(m p) -> p m", p=P))
        bqkv_s = consts.tile([P, 12], F32, tag="bqkv")
        nc.sync.dma_start(out=bqkv_s, in_=d_bqkv.rearrange("(m p) -> p m", p=P))
        ob_s = consts.tile([P, CK], F32, tag="ob")
        nc.sync.dma_start(out=ob_s, in_=d_ob.rearrange("(m p) -> p m", p=P))
        b1_s = consts.tile([P, FFK], F32, tag="b1")
        nc.sync.dma_start(out=b1_s, in_=d_b1.rearrange("(m p) -> p m", p=P))
        bfin_s = consts.tile([P, CK], F32, tag="bfin")
        nc.sync.dma_start(out=bfin_s, in_=d_bfin.rearrange("(m p) -> p m", p=P))
        dsag_s = consts.tile([P, CK], F32, tag="dsag")
        nc.sync.dma_start(out=dsag_s, in_=d_dsag.rearrange("(m p) -> p m", p=P))
        dsab_s = consts.tile([P, CK], F32, tag="dsab")
        nc.sync.dma_start(out=dsab_s, in_=d_dsab.rearrange("(m p) -> p m", p=P))
        dw3_s = consts.tile([P, CK, 3], F32, tag="dw3")
        nc.sync.dma_start(out=dw3_s, in_=d_dw3.rearrange("(m p) d -> p m d", p=P))
        dsadb_s = consts.tile([P, CK], F32, tag="dsadb")
        nc.sync.dma_start(out=dsadb_s, in_=d_dsadb.rearrange("(m p) -> p m", p=P))
        bvbc_s = consts.tile([P, C], F32, tag="bvbc")
        nc.sync.dma_start(out=bvbc_s, in_=d_bvbc[:, :])
        mask2_s = consts.tile([P, 2], F32, tag="mask2")
        nc.sync.dma_start(out=mask2_s, in_=d_mask2[:, :])
        qoff_s = consts.tile([1, 1], U32, tag="qoff")
        nc.sync.dma_start(out=qoff_s, in_=d_qoff[:, :])
        regs = nc.alloc_registers("qoffr")
        nc.regs_load(regs, qoff_s[0:1, 0:1])
        j0 = nc.snap(regs, donate=True, min_val=0, max_val=TL)

        EPS = 1e-5

        vtmp = tc.alloc_tile_pool(name="vtmp", bufs=2, side="left")
        rmr_pool = tc.alloc_tile_pool(name="rmr", bufs=2, side="left")
        tmp_pool = tc.alloc_tile_pool(name="tmp", bufs=2, side="left")
        hat_pool = tc.alloc_tile_pool(name="hatp", bufs=1, side="left")
        hat = hat_pool.tile([P, CK, T], BF, tag="hat")

        def ln_rmr(psS, src_b, src_h, n_cols):
            ps_mean = psS.tile([1, 512], F32, tag="mean")
            for kc in range(CK):
                nc.tensor.matmul(ps_mean[0:1, :n_cols], invC_b[:, :], src_b[kc],
                                 start=(kc == 0), stop=(kc == CK - 1))
            ps_ex2 = psS.tile([1, 512], F32, tag="ex2")
            for kc in range(CK):
                nc.tensor.matmul(ps_ex2[0:1, :n_cols], invC_h[:, :], src_h[kc],
                                 start=(kc == 0), stop=(kc == CK - 1))
            m_s = vtmp.tile([1, 512], F32, tag="m")
            eng.tensor_copy(out=m_s[:, :n_cols], in_=ps_mean[0:1, :n_cols])
            m2_s = vtmp.tile([1, 512], F32, tag="tmpa")
            eng.tensor_tensor(out=m2_s[:, :n_cols], in0=m_s[:, :n_cols],
                              in1=m_s[:, :n_cols], op=OP.mult)
            vpe_s = vtmp.tile([1, 512], F32, tag="tmpa")
            eng.scalar_tensor_tensor(out=vpe_s[:, :n_cols], in0=ps_ex2[0:1, :n_cols],
                                     scalar=EPS, in1=m2_s[:, :n_cols],
                                     op0=OP.add, op1=OP.subtract)
            std_s = vtmp.tile([1, 512], F32, tag="tmpb")
            nc.scalar.activation(out=std_s[:, :n_cols], in_=vpe_s[:, :n_cols],
                                 func=AF.Sqrt)
            r_f = vtmp.tile([1, 512], F32, tag="tmpb")
            eng.reciprocal_approx_fast(out=r_f[:, :n_cols], in_=std_s[:, :n_cols])
            r_s = rmr_pool.tile([1, 512], F32R, tag="r")
            eng.tensor_copy(out=r_s[:, :n_cols], in_=r_f[:, :n_cols])
            mr_s = rmr_pool.tile([1, 512], F32R, tag="mr")
            eng.tensor_tensor(out=mr_s[:, :n_cols], in0=m_s[:, :n_cols],
                              in1=r_f[:, :n_cols], op=OP.mult)
            return r_s, mr_s

        def ln_bcast(psB, r_s, mr_s, n_cols):
            ps_bc = psB.tile([P, 1024], F32, tag="lnbc")
            nc.tensor.matmul(ps_bc[:, 0:n_cols], ones_r[0:1, :],
                             r_s[:, :n_cols], start=True, stop=True)
            nc.tensor.matmul(ps_bc[:, 512:512 + n_cols], ones_r[0:1, :],
                             mr_s[:, :n_cols], start=True, stop=True)
            return ps_bc

        # ============ Fused A1/A2: per chunk conv -> LN1 -> K/V ============
        # elate pool allocated early (stack bottom) but its DMAs issued late
        elate = tc.alloc_tile_pool(name="elate", bufs=1, side="right")
        wo_s = elate.tile([HD, H, C], BF, tag="wo")
        xd_s = elate.tile([P, CK, TL + 2], F32, tag="xd")
        kv_state = tc.alloc_tile_pool(name="kvst", bufs=1, side="right")
        kaug = kv_state.tile([HD + 1, H, T], BF, tag="kaug")
        qaug = kv_state.tile([HD + 1, H, TL], BF, tag="qaug")
        vsb = kv_state.tile([P, TK, H, HD + 1], BF, tag="v")
        st_pool = tc.alloc_tile_pool(name="stage", bufs=3, side="right")
        a2 = tc.alloc_tile_pool(name="a2", bufs=1, side="right")
        wkv_s = a2.tile([P, CK, 2 * C], BF, tag="wkv")
        wq_s = a2.tile([P, CK, C], BF, tag="wq")

        a1 = tc.alloc_tile_pool(name="a1", bufs=1, side="left")
        convw_s = a1.tile([P, 3, CK, C], BF, tag="convw")
        # conv weight + first x chunks first in the DMA stream
        nc.sync.dma_start(
            out=convw_s,
            in_=d_convw.rearrange("d (k p) o -> p d k o", p=P),
        )
        xch_pool = tc.alloc_tile_pool(name="xch", bufs=3, side="left")
        sq_pool = tc.alloc_tile_pool(name="sq", bufs=2, side="left")
        psA = tc.alloc_tile_pool(name="psA", bufs=4, space="PSUM")
        psS = tc.alloc_tile_pool(name="psS", bufs=1, space="PSUM")
        psB = tc.alloc_tile_pool(name="psB", bufs=1, space="PSUM")

        x_tiles = []
        for n in range(NCH):
            c0 = 512 * n
            x_ch = xch_pool.tile([P, CK, 514], BF, tag="xch")
            nc.sync.dma_start(
                out=x_ch,
                in_=d_xT[:, c0:c0 + 514].rearrange("(k p) t -> p k t", p=P),
            )
            x_tiles.append(x_ch)
            if n == 0:
                # K/V/Q weights + aug rows queue behind the first chunk
                nc.sync.dma_start(
                    out=wkv_s,
                    in_=d_wqkv.rearrange("(k p) o -> p k o", p=P)[:, :, C:3 * C],
                )
                nc.sync.dma_start(
                    out=wq_s,
                    in_=d_wqkv.rearrange("(k p) o -> p k o", p=P)[:, :, 0:C],
                )
                eng.tensor_copy(out=vsb[:, :, :, HD],
                                in_=ones_b.rearrange("p (g h) -> p g h", h=H)[:, 0:TK, :])
                for h in range(H):
                    nc.sync.dma_start(out=kaug[HD:HD + 1, h, :], in_=d_A[:, :])
                    nc.sync.dma_start(out=qaug[HD:HD + 1, h, :], in_=d_qA[:, :])

        def conv_chunk(n):
            c0 = 512 * n
            x_ch = x_tiles[n]
            sqs = sq_pool.tile([P, CK, 512], F16, tag="sqs")
            for mo in range(CK):
                ps_c = psA.tile([P, 512], F32, tag="mm")
                first = True
                for dtap in range(3):
                    for kc in range(CK):
                        nc.tensor.matmul(
                            ps_c,
                            convw_s[:, dtap, kc, mo * P:(mo + 1) * P],
                            x_ch[:, kc, dtap:dtap + 512],
                            start=first, stop=(dtap == 2 and kc == CK - 1),
                        )
                        first = False
                f_t = hat[:, mo, c0:c0 + 512]
                nc.scalar.activation(out=f_t, in_=ps_c, func=AF.Gelu,
                                     bias=convb_s[:, mo:mo + 1])
                eng.tensor_tensor(out=f_t, in0=f_t,
                                  in1=x_ch[:, mo, 1:513], op=OP.add)
                nc.scalar.activation(out=sqs[:, mo, :], in_=f_t, func=AF.Square)
            return sqs

        def ln1_chunk(n, sqs):
            c0 = 512 * n
            src_b = [hat[:, kc, c0:c0 + 512] for kc in range(CK)]
            src_h = [sqs[:, kc, :] for kc in range(CK)]
            return ln_rmr(psS, src_b, src_h, 512)

        def ln1_apply(n, r_s, mr_s):
            c0 = 512 * n
            ps_bc = ln_bcast(psB, r_s, mr_s, 512)
            for kc in range(CK):
                t_s = tmp_pool.tile([P, 512], F32, tag="t")
                eng.tensor_tensor(out=t_s, in0=hat[:, kc, c0:c0 + 512],
                                  in1=ps_bc[:, 0:512], op=OP.mult)
                eng.tensor_tensor(out=hat[:, kc, c0:c0 + 512], in0=t_s,
                                  in1=ps_bc[:, 512:1024], op=OP.subtract)

        def kv_chunk(n):
            c0 = 512 * n
            for mo in range(CK):
                ps_k = psA.tile([P, 512], F32, tag="mm")
                for kc in range(CK):
                    nc.tensor.matmul(ps_k, wkv_s[:, kc, mo * P:(mo + 1) * P],
                                     hat[:, kc, c0:c0 + 512],
                                     start=(kc == 0), stop=(kc == CK - 1))
                st = st_pool.tile([P, 512], BF, tag="kst")
                eng.tensor_scalar(out=st, in0=ps_k, scalar1=bqkv_s[:, 4 + mo:5 + mo],
                                  scalar2=None, op0=OP.add)
                nc.sync.dma_start(out=kaug[0:HD, 2 * mo, c0:c0 + 512], in_=st[0:HD, :])
                nc.sync.dma_start(out=kaug[0:HD, 2 * mo + 1, c0:c0 + 512], in_=st[HD:P, :])
            for tt in range(4):
                g = 4 * n + tt
                ps_v = psA.tile([P, 512], F32, tag="mm")
                for kc in range(CK):
                    nc.tensor.matmul(ps_v, hat[:, kc, c0 + tt * P:c0 + (tt + 1) * P],
                                     wkv_s[:, kc, C:2 * C],
                                     start=(kc == 0), stop=(kc == CK - 1))
                eng.tensor_tensor(out=vsb[:, g, :, 0:HD],
                                  in0=ps_v.rearrange("p (h d) -> p h d", d=HD),
                                  in1=bvbc_s.rearrange("p (h d) -> p h d", d=HD),
                                  op=OP.add)

        # software-pipelined: conv(n+1) emitted between stats(n) and bcast(n)
        # so the PE never stalls on the DVE stat chain; K/V(n-1) emitted
        # after bcast(n) so they never stall on apply(n-1).
        sqs_n = conv_chunk(0)
        stats_n = ln1_chunk(0, sqs_n)
        for n in range(NCH):
            if n + 1 < NCH:
                sqs_next = conv_chunk(n + 1)
            ln1_apply(n, *stats_n)
            if n >= 1:
                kv_chunk(n - 1)
            if n + 1 < NCH:
                stats_n = ln1_chunk(n + 1, sqs_next)
        kv_chunk(NCH - 1)
        # late-phase loads queue after all compute-critical DMAs
        nc.sync.dma_start(out=wo_s, in_=d_wo[:, :, :])
        nc.sync.dma_start(out=xd_s,
                          in_=d_xd.rearrange("(k p) t -> p k t", p=P))
        # Q (local half via dynamic offset)
        for mo in range(CK):
            for n2 in range(NL):
                ps_q = psA.tile([P, 512], F32, tag="mm")
                for kc in range(CK):
                    nc.tensor.matmul(ps_q, wq_s[:, kc, mo * P:(mo + 1) * P],
                                     hat[:, kc, bass.ds(j0 + n2 * 512, 512)],
                                     start=(kc == 0), stop=(kc == CK - 1))
                st = st_pool.tile([P, 512], BF, tag="kst")
                eng.tensor_scalar(out=st, in0=ps_q, scalar1=bqkv_s[:, mo:mo + 1],
                                  scalar2=None, op0=OP.add)
                nc.sync.dma_start(out=qaug[0:HD, 2 * mo, n2 * 512:(n2 + 1) * 512],
                                  in_=st[0:HD, :])
                nc.sync.dma_start(out=qaug[0:HD, 2 * mo + 1, n2 * 512:(n2 + 1) * 512],
                                  in_=st[HD:P, :])
        for pool in (sq_pool, xch_pool, a1):
            pool.release()
        for pool in (a2, st_pool, hat_pool, psB, psS, psA):
            pool.release()

        # =================== Attention (software-pipelined) ===============
        attn_state = tc.alloc_tile_pool(name="attnst", bufs=1, side="left")
        attnh = attn_state.tile([HD, H, TL], BF, tag="attnh")
        p_pool = tc.alloc_tile_pool(name="pp", bufs=2, side="right")
        psS2 = tc.alloc_tile_pool(name="psS2", bufs=3, space="PSUM")
        psAV = tc.alloc_tile_pool(name="psAV", bufs=1, space="PSUM")

        p_tiles = [None, None]
        av_tiles = [None, None]

        def normalize(h, ps_av):
            for n2 in range(NL):
                cc = slice(n2 * 512, (n2 + 1) * 512)
                den_s = vtmp.tile([1, 512], F32, tag="tmpb")
                eng.tensor_copy(out=den_s, in_=ps_av[HD:HD + 1, cc])
                d_f = vtmp.tile([1, 512], F32, tag="tmpa")
                eng.reciprocal_approx_fast(out=d_f, in_=den_s)
                d_s = vtmp.tile([1, 512], F32R, tag="d")
                eng.tensor_copy(out=d_s, in_=d_f)
                ps_b = psS2.tile([P, 1024], F32, tag="score")
                nc.tensor.matmul(ps_b[0:HD, 0:512], ones_r[0:1, 0:HD],
                                 d_s, start=True, stop=True)
                db_s = tmp_pool.tile([HD, 512], F32, tag="dbs")
                eng.tensor_copy(out=db_s, in_=ps_b[0:HD, 0:512])
                eng.tensor_tensor(out=attnh[:, h, cc], in0=ps_av[0:HD, cc],
                                  in1=db_s, op=OP.mult)

        for h in range(H + 1):
            if h < H:
                p_tiles[h % 2] = p_pool.tile([P, TK, 1024], BF, tag="p",
                                             name=f"pbuf{h % 2}")
                av_tiles[h % 2] = psAV.tile([HD + 1, 1024], F32, tag="av",
                                            name=f"avbuf{h % 2}")
            for tk in range(TK):
                if h < H:
                    ps_s = psS2.tile([P, 1024], F32, tag="score")
                    for n2 in range(NL):
                        nc.tensor.matmul(ps_s[:, n2 * 512:(n2 + 1) * 512],
                                         kaug[:, h, tk * P:(tk + 1) * P],
                                         qaug[:, h, n2 * 512:(n2 + 1) * 512],
                                         start=True, stop=True)
                    nc.scalar.activation(out=p_tiles[h % 2][:, tk, :], in_=ps_s,
                                         func=AF.Exp)
                if h > 0:
                    pv = p_tiles[(h - 1) % 2]
                    av = av_tiles[(h - 1) % 2]
                    for n2 in range(NL):
                        nc.tensor.matmul(av[:, n2 * 512:(n2 + 1) * 512],
                                         vsb[:, tk, h - 1, :],
                                         pv[:, tk, n2 * 512:(n2 + 1) * 512],
                                         start=(tk == 0), stop=(tk == TK - 1))
            if h > 0:
                normalize(h - 1, av_tiles[(h - 1) % 2])
        for pool in (p_pool, kv_state, psAV, psS2):
            pool.release()

        # =================== late loads ===================
        late = tc.alloc_tile_pool(name="late", bufs=1, side="right")
        xdb_s = late.tile([P, CK, TL + 2], BF, tag="xdb")
        nc.sync.dma_start(out=xdb_s,
                          in_=d_xdb.rearrange("(k p) t -> p k t", p=P))
        ftc2 = late.tile([P, CK, TL], BF, tag="ftc2")
        sq2 = late.tile([P, CK, TL], F16, tag="sq2")
        w1_s = late.tile([P, CK, FF], BF, tag="w1")
        nc.sync.dma_start(out=w1_s,
                          in_=d_w1.rearrange("(k p) o -> p k o", p=P))
        w2_s = late.tile([P, FFK, C], BF, tag="w2")
        nc.sync.dma_start(out=w2_s,
                          in_=d_w2.rearrange("(k p) o -> p k o", p=P))
        pw_s = late.tile([P, CK, C], BF, tag="pw")
        nc.sync.dma_start(out=pw_s,
                          in_=d_pw.rearrange("(k p) o -> p k o", p=P))
        dsa_out = late.tile([P, CK, TL], F32, tag="dsaout")
        z_s = late.tile([P, CK, TL + 2], BF, tag="z")
        z1_s = late.tile([P, CK, TL], BF, tag="z1")
        sqd = late.tile([P, CK, TL + 2], F16, tag="sqd")

        psC = tc.alloc_tile_pool(name="psC", bufs=3, space="PSUM")
        psS_l = tc.alloc_tile_pool(name="psSl", bufs=1, space="PSUM")
        psB_l = tc.alloc_tile_pool(name="psBl", bufs=1, space="PSUM")

        # ---- out-proj + residual (PE) — runs while DSA DVE chain starts
        for mo in range(CK):
            for n2 in range(NL):
                cc = slice(n2 * 512, (n2 + 1) * 512)
                ps_o = psC.tile([P, 512], F32, tag="mm")
                for h in range(H):
                    nc.tensor.matmul(ps_o, wo_s[:, h, mo * P:(mo + 1) * P],
                                     attnh[:, h, cc],
                                     start=(h == 0), stop=(h == H - 1))
                eng.scalar_tensor_tensor(
                    out=ftc2[:, mo, cc], in0=ps_o, scalar=ob_s[:, mo:mo + 1],
                    in1=xd_s[:, mo, 1 + n2 * 512:1 + (n2 + 1) * 512],
                    op0=OP.add, op1=OP.add)
                eng.tensor_tensor(out=sq2[:, mo, cc], in0=ftc2[:, mo, cc],
                                  in1=ftc2[:, mo, cc], op=OP.mult)
        attn_state.release()

        # ---- DSA squares (DVE)
        for kc in range(CK):
            eng.tensor_tensor(out=sqd[:, kc, :], in0=xdb_s[:, kc, :],
                              in1=xdb_s[:, kc, :], op=OP.mult)

        # ---- DSA LN stats+apply, then LN2 stats+apply (sqrts contiguous)
        for (c0, w) in ((0, 512), (512, 512), (1024, 2)):
            src_b = [xdb_s[:, kc, c0:c0 + w] for kc in range(CK)]
            src_h = [sqd[:, kc, c0:c0 + w] for kc in range(CK)]
            r_s, mr_s = ln_rmr(psS_l, src_b, src_h, w)
            ps_bc = ln_bcast(psB_l, r_s, mr_s, w)
            for kc in range(CK):
                t_s = tmp_pool.tile([P, 512], F32, tag="t")
                eng.tensor_tensor(out=t_s[:, :w], in0=xd_s[:, kc, c0:c0 + w],
                                  in1=ps_bc[:, 0:w], op=OP.mult)
                eng.tensor_tensor(out=t_s[:, :w], in0=t_s[:, :w],
                                  in1=ps_bc[:, 512:512 + w], op=OP.subtract)
                eng.tensor_scalar(out=z_s[:, kc, c0:c0 + w], in0=t_s[:, :w],
                                  scalar1=dsag_s[:, kc:kc + 1], scalar2=dsab_s[:, kc:kc + 1],
                                  op0=OP.mult, op1=OP.add)
        for n2 in range(NL):
            cc = slice(n2 * 512, (n2 + 1) * 512)
            src_b = [ftc2[:, kc, cc] for kc in range(CK)]
            src_h = [sq2[:, kc, cc] for kc in range(CK)]
            r_s, mr_s = ln_rmr(psS_l, src_b, src_h, 512)
            ps_bc = ln_bcast(psB_l, r_s, mr_s, 512)
            for kc in range(CK):
                t_s = tmp_pool.tile([P, 512], F32, tag="t")
                eng.tensor_tensor(out=t_s, in0=ftc2[:, kc, cc],
                                  in1=ps_bc[:, 0:512], op=OP.mult)
                eng.tensor_tensor(out=ftc2[:, kc, cc], in0=t_s,
                                  in1=ps_bc[:, 512:1024], op=OP.subtract)
        for pool in (psB_l, psS_l):
            pool.release()
        psO = tc.alloc_tile_pool(name="psO", bufs=1, space="PSUM")

        # ---- DSA depthwise conv + gelu + pointwise
        for i, cpad in enumerate((0, TL + 1)):
            eng.tensor_scalar(out=z_s[:, :, cpad:cpad + 1],
                              in0=z_s[:, :, cpad:cpad + 1],
                              scalar1=mask2_s[:, i:i + 1], scalar2=None, op0=OP.mult)
        for kc in range(CK):
            eng.tensor_scalar(out=z1_s[:, kc, :], in0=z_s[:, kc, 0:TL],
                              scalar1=dw3_s[:, kc, 0:1], scalar2=None, op0=OP.mult)
            eng.scalar_tensor_tensor(out=z1_s[:, kc, :], in0=z_s[:, kc, 1:1 + TL],
                                     scalar=dw3_s[:, kc, 1:2],
                                     in1=z1_s[:, kc, :],
                                     op0=OP.mult, op1=OP.add)
            eng.scalar_tensor_tensor(out=z1_s[:, kc, :], in0=z_s[:, kc, 2:2 + TL],
                                     scalar=dw3_s[:, kc, 2:3],
                                     in1=z1_s[:, kc, :],
                                     op0=OP.mult, op1=OP.add)
            nc.scalar.activation(out=z1_s[:, kc, :], in_=z1_s[:, kc, :],
                                 func=AF.Gelu, bias=dsadb_s[:, kc:kc + 1])
        for mo in range(CK):
            for n2 in range(NL):
                cc = slice(n2 * 512, (n2 + 1) * 512)
                ps_d = psC.tile([P, 512], F32, tag="mm")
                for kc in range(CK):
                    nc.tensor.matmul(ps_d, pw_s[:, kc, mo * P:(mo + 1) * P],
                                     z1_s[:, kc, cc],
                                     start=(kc == 0), stop=(kc == CK - 1))
                eng.tensor_copy(out=dsa_out[:, mo, cc], in_=ps_d)

        # =================== MLP (ff-major, 4 accumulators) ===============
        hh_pool = tc.alloc_tile_pool(name="hh", bufs=3, side="left")
        fin_pool = tc.alloc_tile_pool(name="fin", bufs=3, side="left")
        for n2 in range(NL):
            cc = slice(n2 * 512, (n2 + 1) * 512)
            ps_out = [psO.tile([P, 512], F32, tag=f"out{mo}", name=f"psout{mo}")
                      for mo in range(CK)]
            for ff in range(FFK):
                ps_h = psC.tile([P, 512], F32, tag="mm")
                for kc in range(CK):
                    nc.tensor.matmul(ps_h, w1_s[:, kc, ff * P:(ff + 1) * P],
                                     ftc2[:, kc, cc],
                                     start=(kc == 0), stop=(kc == CK - 1))
                hh_t = hh_pool.tile([P, 512], BF, tag="hh")
                nc.scalar.activation(out=hh_t, in_=ps_h, func=AF.Gelu,
                                     bias=b1_s[:, ff:ff + 1])
                for mo in range(CK):
                    nc.tensor.matmul(ps_out[mo], w2_s[:, ff, mo * P:(mo + 1) * P],
                                     hh_t, start=(ff == 0), stop=(ff == FFK - 1))
            for mo in range(CK):
                fin_t = fin_pool.tile([P, 512], F32, tag="fin")
                eng.scalar_tensor_tensor(out=fin_t, in0=ps_out[mo],
                                         scalar=bfin_s[:, mo:mo + 1],
                                         in1=dsa_out[:, mo, cc],
                                         op0=OP.add, op1=OP.add)
                nc.sync.dma_start(out=d_out[mo * P:(mo + 1) * P, cc], in_=fin_t)

        for pool in (fin_pool, hh_pool, late, tmp_pool, rmr_pool, vtmp,
                     consts, psO, psC):
            pool.release()
        elate.release()

    nc.compile()
    return nc


def _in_maps(inputs):
    f = lambda v: np.ascontiguousarray(np.asarray(v), dtype=np.float32)
    bf = lambda v: np.ascontiguousarray(np.asarray(v, dtype=np.float32).astype(ml_dtypes.bfloat16))
    x = f(inputs["x"])            # [B, T, C]
    A = f(inputs["A"])            # [B, T]
    alpha = float(np.asarray(inputs["alpha_bias"]).reshape(-1)[0])
    dst_a = float(np.asarray(inputs["dst_alpha"]))
    dst_b = float(np.asarray(inputs["dst_beta"]))
    conv1_w, conv1_b = f(inputs["conv1_w"]), f(inputs["conv1_b"])
    ln1_g, ln1_b = f(inputs["ln1_g"]), f(inputs["ln1_b"])
    in_w, in_b = f(inputs["in_proj_w"]), f(inputs["in_proj_b"])
    out_w, out_b = f(inputs["out_w"]), f(inputs["out_b"])
    ln2_g, ln2_b = f(inputs["ln2_g"]), f(inputs["ln2_b"])
    w1, b1 = f(inputs["mlp_w1"]), f(inputs["mlp_b1"])
    w2, b2 = f(inputs["mlp_w2"]), f(inputs["mlp_b2"])
    dsa_g, dsa_b = f(inputs["dsa_ln_g"]), f(inputs["dsa_ln_b"])
    dsa_dw, dsa_db = f(inputs["dsa_dw"]), f(inputs["dsa_db"])
    dsa_pw, dsa_pb = f(inputs["dsa_pw"]), f(inputs["dsa_pb"])

    weff = in_w * ln1_g[None, :]
    beff = in_w @ ln1_b + in_b
    weff[:C] /= np.sqrt(HD).astype(np.float32)
    beff[:C] /= np.sqrt(HD).astype(np.float32)
    shared = {
        "convw": bf(np.transpose(conv1_w, (2, 1, 0))),
        "convb": conv1_b,
        "wqkv": bf(weff.T),
        "bqkv": beff,
        "bvbc": np.ascontiguousarray(np.broadcast_to(beff[2 * C:3 * C], (P, C))),
        "wo": bf(out_w.T.reshape(HD * H, C).reshape(H, HD, C).transpose(1, 0, 2)),
        "ob": out_b,
        "w1": bf((w1 * ln2_g[None, :]).T),
        "b1": w1 @ ln2_b + b1,
        "w2": bf((dst_a * w2).T),
        "bfin": dst_a * b2 + dst_b * dsa_pb,
        "pw": bf((dst_b * dsa_pw[:, :, 0]).T),
        "dsag": dsa_g, "dsab": dsa_b,
        "dw3": dsa_dw[:, 0, :], "dsadb": dsa_db,
        "cones": np.ones((P, P), np.float32),
        "conesb": np.ones((P, P), ml_dtypes.bfloat16),
        "cinvC": np.full((P, 1), 1.0 / C, ml_dtypes.bfloat16),
        "cinvCh": np.full((P, 1), 1.0 / C, np.float16),
    }
    maps = []
    for core in range(8):
        b, half = core // 2, core % 2
        j0 = half * TL
        xT = np.zeros((C, T + 2), np.float32)
        xT[:, 1:T + 1] = x[b].T
        xd = np.zeros((C, TL + 2), np.float32)
        lo, hi = j0 - 1, j0 + TL + 1
        slo, shi = max(lo, 0), min(hi, T)
        xd[:, slo - lo:slo - lo + (shi - slo)] = x[b].T[:, slo:shi]
        m = dict(shared)
        m["xT"] = bf(xT)
        m["xd"] = xd
        m["xdb"] = bf(xd)
        mask2 = np.ones((P, 2), np.float32)
        if lo < 0:
            mask2[:, 0] = 0.0
        if hi > T:
            mask2[:, 1] = 0.0
        m["mask2"] = mask2
        m["Arow"] = bf(A[b:b + 1, :])
        m["qArow"] = bf(alpha * A[b:b + 1, j0:j0 + TL])
        m["qoff"] = np.array([[j0]], np.uint32)
        maps.append(m)
    return maps


def _get_program():
    global _CACHED
    if _CACHED is None:
        _CACHED = _build()
    return _CACHED


def kernel(**inputs):
    nc = _get_program()
    maps = _in_maps(inputs)
    res = run_bass_kernel_spmd(nc, maps, list(range(8)))
    out = np.empty((B, T, C), np.float32)
    for core in range(8):
        b, half = core // 2, core % 2
        out[b, half * TL:(half + 1) * TL, :] = res.results[core]["outT"].T
    return out
